# revision 1
# baseline (speedup 1.0000x reference)
"""Trainium2 Bass kernel for nn_AnchorDeformAtt (deformable anchor attention).

Sharding: spatial L-shard across 8 cores -- core i handles pixels
l in [512i, 512(i+1)) for BOTH batches and ALL heads. Zero collectives;
the host concatenates per-core output shards.

Design:
  - Memory: quad rows (m[j], m[j+1], m[j+64], m[j+65]) bf16, so ONE
    index per sample point fetches all 4 bilinear taps (d=4). 8
    ap_gathers per core (one per (b, hg, l-block), num_idxs=4096):
    minimal index count AND one input-scan charge per block.
  - Value conv in bf16 (feat staged bf16 host-side); bias folded out
    (softmax x bilinear weights sum to 1 => Wout @ bv is a constant
    output bias, merged with bn_beta host-side; BN scale folded into
    Wout). Prep convs in float32r (tf32 rate).
  - Gather stream per head 16-partition group: i = (p, lhi, lq) with
    row q = lq = l%16, cols (p, lhi). Indices are wrapped at STORE
    time into DRAM [k][hg][b][h][lq][p][lhi] (the strided store pays
    the transpose tax once); idxw loads are clean 2-dim DMAs.
  - u = attn * bilinear weights staged as [b][(h,p)][l][dx4],
    broadcast to each head's 32 channels with 3-dim-AP DMAs.
  - Combine matmuls (contraction over 4 heads x 32 ch, stride-4 rhs
    per (p, dx)) accumulate psum[o, 256l] over (hg, hf, p, dx); one
    ACT copy + DMA out per (b, lblk, oc).
  - Emission interleaves prep phases between gather blocks so the
    gather engine never starves. The backend serializes DMA per issuing
    queue, so the bulk traffic (u-broadcast, feature loads) is
    round-robined across the SP / ACT / GPSIMD queues to balance all
    three at ~250us each.
"""
from contextlib import ExitStack

import numpy as np
import ml_dtypes

import concourse.bass as bass
import concourse.mybir as mybir
import concourse.tile as tile
from concourse import bacc
from concourse.bass_utils import run_bass_kernel_spmd

NH, NP = 8, 16
B, C, H, W = 2, 256, 64, 64
L = H * W            # 4096
NCORES = 8
LSH = L // NCORES    # 512
LPAD = L + 64        # pairs-memory rows (y1 tap reads idx+64)
EPS = 1e-6
F32 = mybir.dt.float32
BF16 = mybir.dt.bfloat16
I16 = mybir.dt.int16
F32R = mybir.dt.float32r
FP8 = mybir.dt.float8e4

_GRAPH_CACHE = {}

PARAM_SPECS = {
    'featb': ([128, B, 2, L], BF16),
    'fsh': ([128, B, 2, LSH], F32R),
    'wv_t': ([128, 2, 2, 128], BF16),    # K, hg, kc, M
    'woff_t': ([128, 2, 2, 128], F32R),   # K, xy, kc, M
    'boff_p': ([128, 2], F32),           # per-partition bias, xy
    'wsz_t': ([128, 2, 2, 8], F32R),      # K, xy, kc, 8
    'bsz_p': ([8, 2], F32),
    'sel8': ([8, 128], F32),             # sel8[h, h*16+p] = 1
    'watt_t': ([128, 2, 128], F32R),      # K, kc, N
    'batt_r': ([1, 128], F32),
    'ones1': ([1, 128], F32),
    'ident': ([128, 128], F32),
    'wout_t': ([128, 2, 2, 128], BF16),  # K, hg, oc, M (bn-scaled)
    'obias': ([128, 2], F32),            # (Wout_sc @ bv + beta) as [m, oc]
    'cen2': ([128, 2, LSH], F32),        # packed (x|y) centers
}


def build_graph(stub_gather=False):
    key = (stub_gather,)
    if key in _GRAPH_CACHE:
        return _GRAPH_CACHE[key]

    nc = bacc.Bacc("TRN2", target_bir_lowering=False, debug=False,
                   num_devices=NCORES)
    dp = nc.declare_dram_parameter
    P = {n: dp(n, s, dt, isOutput=False) for n, (s, dt) in PARAM_SPECS.items()}
    out_e = dp("out", [B, 2, 128, LSH], F32, isOutput=True)

    # wrapped y0 idx staging, one tensor per (k, hg): [b][h4][lq][p][lhi]
    idxd = {(k, hg): nc.dram_tensor(f"idxd{k}{hg}", [B, 4, 16, 16, 16], I16)
            for k in range(2) for hg in range(2)}
    # u staging: [b][(h,p) 128][tap 2][l 512][dx 2]
    ud = nc.dram_tensor("ud", [B, 128, 2, LSH, 2], BF16)

    AP = bass.AP
    Act = mybir.ActivationFunctionType
    Alu = mybir.AluOpType

    with tile.TileContext(nc) as tc, ExitStack() as ctx:
        consts = ctx.enter_context(tc.tile_pool(name="consts", bufs=1))
        featp = ctx.enter_context(tc.tile_pool(name="featp", bufs=2))
        fshp = ctx.enter_context(tc.tile_pool(name="fshp", bufs=1))
        memp = ctx.enter_context(tc.tile_pool(name="memp", bufs=1))
        prep = ctx.enter_context(tc.tile_pool(name="prep", bufs=1))
        gm = ctx.enter_context(tc.tile_pool(name="gm", bufs=1))
        idxwp = ctx.enter_context(tc.tile_pool(name="idxwp", bufs=2))
        ubcp = ctx.enter_context(tc.tile_pool(name="ubcp", bufs=2))
        gathp = ctx.enter_context(tc.tile_pool(name="gathp", bufs=2))
        outp = ctx.enter_context(tc.tile_pool(name="outp", bufs=2))
        ps_v = ctx.enter_context(tc.tile_pool(name="ps_v", bufs=2, space="PSUM"))
        ps_p = ctx.enter_context(tc.tile_pool(name="ps_p", bufs=2, space="PSUM"))
        ps_o = ctx.enter_context(tc.tile_pool(name="ps_o", bufs=1, space="PSUM"))

        def dmas(out, in_):          # SP queue: consts, ft, idx stores, ubc
            nc.sync.dma_start(out=out, in_=in_)

        def dmaa(out, in_):          # ACT queue: fsh, idxw loads, uq, out
            nc.scalar.dma_start(out=out, in_=in_)

        # ---------------- constants ----------------
        def cload(name, q=None):
            shape, dt = PARAM_SPECS[name]
            t = consts.tile(list(shape), dt, tag=name, name=f"c_{name}")
            (q or dmas)(t[:], P[name].ap())
            return t

        def dmag(out, in_):
            nc.gpsimd.dma_start(out=out, in_=in_)

        wv_sb = cload('wv_t')
        woff_sb = cload('woff_t')
        boff_sb = cload('boff_p')
        wsz_sb = cload('wsz_t')
        bsz_sb = cload('bsz_p')
        sel8_sb = cload('sel8')
        watt_sb = cload('watt_t')
        batt_sb = cload('batt_r')
        ones_sb = cload('ones1')
        id_sb = cload('ident')
        wout_sb = cload('wout_t')
        obias_sb = cload('obias')
        cen_sb = cload('cen2')

        mem_t = {}
        st = {}

        # ---------------- value conv (bf16 quads, no bias) ----------------
        def conv_sec(b, hgs=(0, 1)):
            qs = {}
            for hg in hgs:
                q = memp.tile([128, L, 4], BF16, tag="quad",
                              name=f"quad{b}{hg}", bufs=2)
                mem_t[(b, hg)] = q
                nc.vector.memset(q[:, L - 65:, :], 0.0)
                qs[hg] = q
            for c in range(4):
                ft = featp.tile([128, 2, 1024], BF16, tag="ft")
                ftq = (dmas, dmaa,
                       lambda o, i: nc.gpsimd.dma_start(out=o, in_=i))[c % 3]
                ftq(ft[:], AP(tensor=P['featb'], offset=b * 2 * L + c * 1024,
                              ap=[[B * 2 * L, 128], [L, 2], [1, 1024]]))
                for j in range(2):
                    n = c * 2 + j
                    for hg in hgs:
                        q = qs[hg]
                        ps = ps_v.tile([128, 512], F32, tag="pv")
                        for kc in range(2):
                            nc.tensor.matmul(ps[:], wv_sb[:, hg, kc, :],
                                             ft[:, kc, j * 512:(j + 1) * 512],
                                             start=(kc == 0), stop=(kc == 1))
                        for dxi, sh in ((0, 0), (1, 1), (2, 64), (3, 65)):
                            if sh == 0:
                                o, i = q[:, n * 512:(n + 1) * 512, 0], ps[:]
                            elif n == 0:
                                o, i = q[:, 0:512 - sh, dxi], ps[:, sh:512]
                            else:
                                o = q[:, n * 512 - sh:(n + 1) * 512 - sh, dxi]
                                i = ps[:]
                            if dxi < 2:
                                nc.scalar.activation(out=o, in_=i,
                                                     func=Act.Copy)
                            else:
                                nc.vector.tensor_copy(out=o, in_=i)

        # ---------------- prep phases (per b) ----------------
        def prep_a(b):
            """fsh load; offset+size convs -> packed offp/szbp [128,2,LSH]."""
            s = st.setdefault(b, {})
            fsh = fshp.tile([128, 2, LSH], F32R, tag="fsh")
            dmaa(fsh[:], P['fsh'].ap()[:, b, :, :])
            s['fsh'] = fsh
            offp = prep.tile([128, 2, LSH], F32, tag="offp")
            szbp = prep.tile([128, 2, LSH], F32, tag="szbp")
            s['offp'], s['szbp'] = offp, szbp
            for xy in range(2):
                ps = ps_p.tile([128, 512], F32, tag="pp", name="psz")
                for kc in range(2):
                    nc.tensor.matmul(ps[0:8, :], wsz_sb[:, xy, kc, :],
                                     fsh[:, kc, :], start=(kc == 0),
                                     stop=(kc == 1))
                szs = gm.tile([8, LSH], F32, tag="szs")
                nc.scalar.activation(out=szs[:], in_=ps[0:8, :],
                                     func=Act.Sigmoid,
                                     bias=bsz_sb[:, xy:xy + 1], scale=1.0)
                nc.vector.tensor_scalar(out=szs[:], in0=szs[:], scalar1=0.75,
                                        scalar2=0.25, op0=Alu.min, op1=Alu.max)
                psb = ps_p.tile([128, 512], F32, tag="pp", name="psb")
                nc.tensor.matmul(psb[:], sel8_sb[:], szs[:],
                                 start=True, stop=True)
                nc.vector.tensor_copy(out=szbp[:, xy, :], in_=psb[:])
                ps2 = ps_p.tile([128, 512], F32, tag="pp", name="po")
                for kc in range(2):
                    nc.tensor.matmul(ps2[:], woff_sb[:, xy, kc, :],
                                     fsh[:, kc, :], start=(kc == 0),
                                     stop=(kc == 1))
                nc.scalar.activation(out=offp[:, xy, :], in_=ps2[:],
                                     func=Act.Sigmoid,
                                     bias=boff_sb[:, xy:xy + 1], scale=1.0)

        def prep_c(b):
            """grid -> floor -> flat y0 idx (packed x|y in one [128,2,LSH])."""
            s = st[b]
            offp, szbp = s['offp'], s['szbp']
            o2 = offp[:].rearrange("p a b -> p (a b)")
            s2 = szbp[:].rearrange("p a b -> p (a b)")
            cf = gm.tile([128, 2, LSH], F32, tag="cf")
            c2 = cf[:].rearrange("p a b -> p (a b)")
            ci = gm.tile([128, 2, LSH], I16, tag="ci")
            i2 = ci[:].rearrange("p a b -> p (a b)")
            msk = gm.tile([128, 2, LSH], F32, tag="msk")
            m2 = msk[:].rearrange("p a b -> p (a b)")
            nc.vector.tensor_scalar(out=o2, in0=o2, scalar1=-0.5,
                                    scalar2=None, op0=Alu.add)
            nc.vector.tensor_tensor(out=o2, in0=o2, in1=s2, op=Alu.mult)
            nc.vector.tensor_tensor(
                out=o2, in0=o2,
                in1=cen_sb[:].rearrange("p a b -> p (a b)"), op=Alu.add)
            nc.vector.tensor_scalar(out=o2, in0=o2, scalar1=float(W - 1),
                                    scalar2=0.0, op0=Alu.min, op1=Alu.max)
            nc.vector.tensor_copy(out=i2, in_=o2)
            nc.vector.tensor_copy(out=c2, in_=i2)
            nc.vector.tensor_tensor(out=m2, in0=c2, in1=o2, op=Alu.is_gt)
            nc.vector.tensor_tensor(out=c2, in0=c2, in1=m2, op=Alu.subtract)
            nc.vector.tensor_tensor(out=o2, in0=o2, in1=c2, op=Alu.subtract)
            # flat y0 = y0f*W + x0f -> reuse szbp x-half as scratch, fi -> ci
            fl = szbp[:, 0, :]
            nc.vector.tensor_scalar(out=fl, in0=cf[:, 1, :],
                                    scalar1=float(W), scalar2=None,
                                    op0=Alu.mult)
            nc.vector.tensor_tensor(out=fl, in0=fl, in1=cf[:, 0, :],
                                    op=Alu.add)
            fi = gm.tile([128, LSH], I16, tag="fi")
            nc.vector.tensor_copy(out=fi[:], in_=fl)
            s['fi'] = fi
            # wx/wy in offp halves; cf/msk slots free for prep_d reuse
            s['cf'], s['msk'] = cf, msk

        def prep_cs(b, k):
            """Wrapped y0 idx stores for l-block k + clean idxw loads +
            DVE-derived y1 idx tiles."""
            s = st[b]
            fi = s['fi']
            for hg in range(2):
                for hh in range(4):
                    h = hg * 4 + hh
                    dmas(AP(tensor=idxd[(k, hg)], offset=(b * 4 + hh) * 4096,
                            ap=[[16, 16], [1, 16], [256, 16]]),
                         fi[h * 16:(h + 1) * 16, k * 256:(k + 1) * 256])
                ix = idxwp.tile([128, 256], I16, tag=f"ix{hg}{k}",
                                name=f"ix{b}{hg}{k}")
                st[('ix', b, hg, k)] = ix
                for hh in range(4):
                    for dup in range(2):
                        r = hh * 32 + dup * 16
                        dmaa(ix[r:r + 16, :],
                             AP(tensor=idxd[(k, hg)],
                                offset=(b * 4 + hh) * 4096,
                                ap=[[256, 16], [1, 256]]))

        def prep_b(b):
            """attn conv (pixel-major) + softmax + transpose -> aT."""
            s = st[b]
            fsh = s['fsh']
            aT = prep.tile([128, LSH], F32, tag="aT")
            s['aT'] = aT
            for lb in range(LSH // 128):
                ps = ps_p.tile([128, 128], F32, tag="pp", name="pa")
                for kc in range(2):
                    nc.tensor.matmul(ps[:], fsh[:, kc, lb * 128:(lb + 1) * 128],
                                     watt_sb[:, kc, :], start=(kc == 0),
                                     stop=False)
                nc.tensor.matmul(ps[:], ones_sb[:], batt_sb[:],
                                 start=False, stop=True)
                ae = gm.tile([128, 8, 16], F32, tag="ae")
                nc.scalar.activation(out=ae[:], in_=ps[:], func=Act.Exp)
                ssum = gm.tile([128, 8, 1], F32, tag="ssum")
                nc.vector.tensor_reduce(out=ssum[:], in_=ae[:],
                                        axis=mybir.AxisListType.X, op=Alu.add)
                nc.vector.reciprocal(out=ssum[:], in_=ssum[:])
                for h in range(NH):
                    nc.vector.tensor_scalar(out=ae[:, h, :], in0=ae[:, h, :],
                                            scalar1=ssum[:, h, :],
                                            scalar2=None, op0=Alu.mult)
                pst = ps_p.tile([128, 128], F32, tag="pp", name="pt")
                nc.tensor.transpose(pst[:], ae[:].rearrange("p a b -> p (a b)"),
                                    id_sb[:])
                nc.scalar.activation(out=aT[:, lb * 128:(lb + 1) * 128],
                                     in_=pst[:], func=Act.Copy)

        def prep_d(b):
            """u = attn * bilinear -> upair [128, tap, l, dx] -> DRAM."""
            s = st[b]
            offp, cf, msk, aT = s['offp'], s['cf'], s['msk'], s['aT']
            wx, wy = offp[:, 0, :], offp[:, 1, :]
            omx, omy = msk[:, 0, :], msk[:, 1, :]
            ay0, ay1 = cf[:, 0, :], cf[:, 1, :]
            nc.vector.tensor_scalar(out=omx, in0=wx, scalar1=-1.0,
                                    scalar2=1.0, op0=Alu.mult, op1=Alu.add)
            nc.vector.tensor_scalar(out=omy, in0=wy, scalar1=-1.0,
                                    scalar2=1.0, op0=Alu.mult, op1=Alu.add)
            nc.vector.tensor_tensor(out=ay0, in0=aT[:], in1=omy, op=Alu.mult)
            nc.vector.tensor_tensor(out=ay1, in0=aT[:], in1=wy, op=Alu.mult)
            uq = gm.tile([128, LSH, 4], BF16, tag="uq")
            for dxi, (yf, xf) in enumerate(((ay0, omx), (ay0, wx),
                                            (ay1, omx), (ay1, wx))):
                nc.vector.tensor_tensor(out=uq[:, :, dxi], in0=yf,
                                        in1=xf, op=Alu.mult)
            dmaa(AP(tensor=ud, offset=b * 128 * 2048,
                    ap=[[2048, 128], [1, 2048]]),
                 uq[:].rearrange("p a b -> p (a b)"))

        # ---------------- gather + combine ----------------
        pso = {}
        cnt = {}
        ubc_rr = [0]

        def gblock(b, hg, k):
            quad = mem_t[(b, hg)]
            idxw = st[('ix', b, hg, k)]
            for oc in range(2):
                if (b, k, oc) not in pso:
                    pso[(b, k, oc)] = ps_o.tile([128, 256], F32,
                                                tag=f"po{k}{oc}",
                                                name=f"po{b}{k}{oc}")
                    cnt[(b, k, oc)] = 0
            g = gathp.tile([128, 4096, 4], BF16, tag="g4")
            if stub_gather:
                nc.gpsimd.ap_gather(
                    g[:, 0:16, :], quad[:].rearrange("p a b -> p (a b)"),
                    idxw[:, 0:1], channels=128,
                    num_elems=L, d=4, num_idxs=16)
            else:
                nc.gpsimd.ap_gather(
                    g[:], quad[:].rearrange("p a b -> p (a b)"),
                    idxw[:], channels=128,
                    num_elems=L, d=4, num_idxs=4096)
            for jq in range(4):       # quarter u-broadcasts: p in [4jq, 4jq+4)
                ubc = ubcp.tile([128, 4096], BF16, tag="ubc")
                for hh in range(4):
                    # balance broadcast traffic across SP/ACT/Pool DMA queues
                    r = ubc_rr[0] % 11
                    ubc_rr[0] += 1
                    dmaq = (dmas if r < 5 else
                            dmaa if r < 8 else
                            (lambda o, i: nc.gpsimd.dma_start(out=o, in_=i)))
                    dmaq(ubc[hh * 32:(hh + 1) * 32, :],
                         AP(tensor=ud,
                            offset=(b * 128 + (hg * 4 + hh) * 16 + jq * 4)
                            * 2048 + k * 1024,
                            ap=[[0, 32], [2048, 4], [1, 1024]]))
                nc.vector.tensor_tensor(
                    out=g[:, jq * 1024:(jq + 1) * 1024, :].rearrange(
                        "p a b -> p (a b)"),
                    in0=g[:, jq * 1024:(jq + 1) * 1024, :].rearrange(
                        "p a b -> p (a b)"),
                    in1=ubc[:], op=Alu.mult)
            gap = g[:]
            for oc in range(2):
                for p in range(16):
                    for dxi in range(4):
                        rhs = AP(tensor=gap.tensor,
                                 offset=gap.offset + p * 1024 + dxi,
                                 ap=[gap.ap[0], [4, 256]])
                        c = cnt[(b, k, oc)]
                        nc.tensor.matmul(
                            pso[(b, k, oc)][:],
                            wout_sb[:, hg, oc, :], rhs,
                            start=(c == 0), stop=(c == 127))
                        cnt[(b, k, oc)] = c + 1

        def finalize(b, k):
            for oc in range(2):
                o_sb = outp.tile([128, 256], F32, tag="osb")
                nc.scalar.activation(out=o_sb[:], in_=pso[(b, k, oc)][:],
                                     func=Act.Identity,
                                     bias=obias_sb[:, oc:oc + 1], scale=1.0)
                dmaa(AP(tensor=out_e,
                        offset=((b * 2 + oc) * 128) * LSH + k * 256,
                        ap=[[LSH, 128], [1, 256]]), o_sb[:])

        # ---------------- emission schedule ----------------
        prep_a(0)
        prep_c(0)
        conv_sec(0)
        prep_cs(0, 0)
        prep_b(0)
        prep_d(0)
        gblock(0, 0, 0)
        prep_cs(0, 1)
        gblock(0, 0, 1)
        prep_a(1)
        prep_b(1)
        prep_c(1)
        prep_d(1)
        gblock(0, 1, 0)
        finalize(0, 0)
        prep_cs(1, 0)
        gblock(0, 1, 1)
        finalize(0, 1)
        prep_cs(1, 1)
        conv_sec(1, (0,))
        conv_sec(1, (1,))
        gblock(1, 0, 0)
        gblock(1, 0, 1)
        gblock(1, 1, 0)
        finalize(1, 0)
        gblock(1, 1, 1)
        finalize(1, 1)

    nc.compile()
    _GRAPH_CACHE[key] = nc
    return nc


def stage_inputs(inputs, core):
    """Build the per-core in_map (all arrays pre-laid-out for plain DMAs)."""
    bf16 = ml_dtypes.bfloat16
    feat = np.ascontiguousarray(
        np.asarray(inputs['feat_sd'], np.float32).reshape(B, C, L))
    lo = core * LSH
    WvT = np.asarray(inputs['value_proj_w'], np.float32).T.copy()
    WoffT = np.asarray(inputs['anchor_deform_w'], np.float32).T.copy()
    WattT = np.asarray(inputs['anchor_att_w'], np.float32).T.copy()
    WszT = np.asarray(inputs['size_deform_w'], np.float32).T.copy()
    WoutT = np.asarray(inputs['out_proj_w'], np.float32).T.copy()
    boff = np.asarray(inputs['anchor_deform_b'], np.float32)
    bsz = np.asarray(inputs['size_deform_b'], np.float32)
    bv = np.asarray(inputs['value_proj_b'], np.float32)
    bn_s = (np.asarray(inputs['bn_gamma'], np.float32)
            / np.sqrt(np.float32(1.0 + 1e-5)))
    beta = np.asarray(inputs['bn_beta'], np.float32)
    WoutT_sc = WoutT * bn_s[None, :]
    obias = (bv @ WoutT_sc + beta).reshape(2, 128).T
    sel8 = np.zeros((8, 128), np.float32)
    for h in range(8):
        sel8[h, h * 16:(h + 1) * 16] = float(W - 1)
    cols = (np.arange(W) + 0.5) / (W + EPS)
    rows = (np.arange(H) + 0.5) / (H + EPS)
    cx = np.tile(cols, H)[lo:lo + LSH].astype(np.float32)
    cy = np.repeat(rows, W)[lo:lo + LSH].astype(np.float32)
    cen2 = np.stack([np.broadcast_to(cx, (128, LSH)),
                     np.broadcast_to(cy, (128, LSH))], axis=1) * (W - 1.0)
    # woff/wsz packed: xy-interleaved output channels split into x|y planes
    woff = np.stack([WoffT[:, 0::2], WoffT[:, 1::2]],
                    axis=1)                      # [256, 2, 128]
    woff_t = woff.reshape(2, 128, 2, 128).transpose(1, 2, 0, 3)
    wsz = np.stack([WszT[:, 0::2], WszT[:, 1::2]], axis=1)  # [256, 2, 8]
    wsz_t = wsz.reshape(2, 128, 2, 8).transpose(1, 2, 0, 3)
    fr = feat.reshape(B, 2, 128, L)
    m = {
        'featb': np.ascontiguousarray(
            fr.transpose(2, 0, 1, 3)).astype(bf16),
        'fsh': np.ascontiguousarray(
            fr[:, :, :, lo:lo + LSH].transpose(2, 0, 1, 3)),
        'wv_t': np.ascontiguousarray(
            WvT.reshape(2, 128, 2, 128).transpose(1, 2, 0, 3)).astype(bf16),
        'woff_t': np.ascontiguousarray(woff_t),
        'boff_p': np.ascontiguousarray(
            np.stack([boff[0::2], boff[1::2]], axis=1)),
        'wsz_t': np.ascontiguousarray(wsz_t),
        'bsz_p': np.ascontiguousarray(
            np.stack([bsz[0::2], bsz[1::2]], axis=1)),
        'sel8': sel8,
        'watt_t': np.ascontiguousarray(
            WattT.reshape(2, 128, 128).transpose(1, 0, 2)),
        'batt_r': np.asarray(inputs['anchor_att_b'],
                             np.float32).reshape(1, 128),
        'ones1': np.ones((1, 128), np.float32),
        'ident': np.eye(128, dtype=np.float32),
        'wout_t': np.ascontiguousarray(
            WoutT_sc.reshape(2, 128, 2, 128).transpose(1, 0, 2, 3)
        ).astype(bf16),
        'obias': np.ascontiguousarray(obias),
        'cen2': np.ascontiguousarray(cen2),
    }
    return m


def kernel(**inputs):
    nc = build_graph()
    in_maps = [stage_inputs(inputs, i) for i in range(NCORES)]
    res = run_bass_kernel_spmd(nc, in_maps, core_ids=list(range(NCORES)))
    shards = [res.results[i]['out'].reshape(B, C, LSH) for i in range(NCORES)]
    full = np.concatenate(shards, axis=2).reshape(B, C, H, W)
    return full.astype(np.float32)



# revision 10
# speedup vs baseline: 1.0418x; 1.0418x over previous
"""Trainium2 Bass kernel for nn_AnchorDeformAtt (deformable anchor attention).

Sharding: spatial L-shard across 8 cores -- core i handles pixels
l in [512i, 512(i+1)) for BOTH batches and ALL heads. Zero collectives;
the host concatenates per-core output shards.

Design:
  - Memory: quad rows (m[j], m[j+1], m[j+64], m[j+65]) bf16, so ONE
    index per sample point fetches all 4 bilinear taps (d=4). 8
    ap_gathers per core (one per (b, hg, l-block), num_idxs=4096):
    minimal index count AND one input-scan charge per block.
  - Value conv in bf16 (feat staged bf16 host-side); bias folded out
    (softmax x bilinear weights sum to 1 => Wout @ bv is a constant
    output bias, merged with bn_beta host-side; BN scale folded into
    Wout). Prep convs in float32r (tf32 rate).
  - Gather stream per head 16-partition group: i = (p, lhi, lq) with
    row q = lq = l%16, cols (p, lhi). Indices are wrapped at STORE
    time into DRAM [k][hg][b][h][lq][p][lhi] (the strided store pays
    the transpose tax once); idxw loads are clean 2-dim DMAs.
  - u = attn * bilinear weights staged as [b][(h,p)][l][dx4],
    broadcast to each head's 32 channels with 3-dim-AP DMAs.
  - Combine matmuls (contraction over 4 heads x 32 ch, stride-4 rhs
    per (p, dx)) accumulate psum[o, 256l] over (hg, hf, p, dx); one
    ACT copy + DMA out per (b, lblk, oc).
  - Emission interleaves prep phases between gather blocks so the
    gather engine never starves. The backend serializes DMA per issuing
    queue, so the bulk traffic (u-broadcast, feature loads) is
    round-robined across the SP / ACT / GPSIMD queues to balance all
    three at ~250us each.
"""
from contextlib import ExitStack

import numpy as np
import ml_dtypes

import concourse.bass as bass
import concourse.mybir as mybir
import concourse.tile as tile
from concourse import bacc
from concourse.bass_utils import run_bass_kernel_spmd

NH, NP = 8, 16
B, C, H, W = 2, 256, 64, 64
L = H * W            # 4096
NCORES = 8
LSH = L // NCORES    # 512
LPAD = L + 64        # pairs-memory rows (y1 tap reads idx+64)
EPS = 1e-6
F32 = mybir.dt.float32
BF16 = mybir.dt.bfloat16
I16 = mybir.dt.int16
F32R = mybir.dt.float32r
FP8 = mybir.dt.float8e4

_GRAPH_CACHE = {}

PARAM_SPECS = {
    'featb': ([128, B, 2, L], BF16),
    'fsh': ([128, B, 2, LSH], F32R),
    'wv_t': ([128, 2, 2, 128], BF16),    # K, hg, kc, M
    'woff_t': ([128, 2, 2, 128], F32R),   # K, xy, kc, M
    'boff_p': ([128, 2], F32),           # per-partition bias, xy
    'wsz_t': ([128, 2, 2, 8], F32R),      # K, xy, kc, 8
    'bsz_p': ([8, 2], F32),
    'sel8': ([8, 128], F32),             # sel8[h, h*16+p] = 1
    'watt_t': ([128, 2, 128], F32R),      # K, kc, N
    'batt_r': ([1, 128], F32),
    'ones1': ([1, 128], F32),
    'ident': ([128, 128], F32),
    'wout_t': ([128, 2, 2, 128], BF16),  # K, hg, oc, M (bn-scaled)
    'obias': ([128, 2], F32),            # (Wout_sc @ bv + beta) as [m, oc]
    'cen2': ([128, 2, LSH], F32),        # packed (x|y) centers
}


def build_graph(stub_gather=False):
    key = (stub_gather,)
    if key in _GRAPH_CACHE:
        return _GRAPH_CACHE[key]

    nc = bacc.Bacc("TRN2", target_bir_lowering=False, debug=False,
                   num_devices=NCORES)
    dp = nc.declare_dram_parameter
    P = {n: dp(n, s, dt, isOutput=False) for n, (s, dt) in PARAM_SPECS.items()}
    out_e = dp("out", [B, 2, 128, LSH], F32, isOutput=True)

    # wrapped y0 idx staging, one tensor per (k, hg): [b][lq][hh][p][lhi]
    # (lq-major so the (hh,p) partition dims merge to one stride-16 dim)
    idxd = {(k, hg): nc.dram_tensor(f"idxd{k}{hg}", [B, 16, 4, 16, 16], I16)
            for k in range(2) for hg in range(2)}
    # u staging: [b][(h,p) 128][tap 2][l 512][dx 2]
    ud = nc.dram_tensor("ud", [B, 128, 2, LSH, 2], BF16)

    AP = bass.AP
    Act = mybir.ActivationFunctionType
    Alu = mybir.AluOpType

    with tile.TileContext(nc) as tc, ExitStack() as ctx:
        consts = ctx.enter_context(tc.tile_pool(name="consts", bufs=1))
        featp = ctx.enter_context(tc.tile_pool(name="featp", bufs=2))
        fshp = ctx.enter_context(tc.tile_pool(name="fshp", bufs=1))
        memp = ctx.enter_context(tc.tile_pool(name="memp", bufs=1))
        prep = ctx.enter_context(tc.tile_pool(name="prep", bufs=1))
        gm = ctx.enter_context(tc.tile_pool(name="gm", bufs=1))
        idxwp = ctx.enter_context(tc.tile_pool(name="idxwp", bufs=2))
        ubcp = ctx.enter_context(tc.tile_pool(name="ubcp", bufs=2))
        gathp = ctx.enter_context(tc.tile_pool(name="gathp", bufs=2))
        outp = ctx.enter_context(tc.tile_pool(name="outp", bufs=2))
        ps_v = ctx.enter_context(tc.tile_pool(name="ps_v", bufs=2, space="PSUM"))
        ps_p = ctx.enter_context(tc.tile_pool(name="ps_p", bufs=2, space="PSUM"))
        ps_o = ctx.enter_context(tc.tile_pool(name="ps_o", bufs=1, space="PSUM"))

        def dmas(out, in_):          # SP queue: consts, ft, idx stores, ubc
            nc.sync.dma_start(out=out, in_=in_)

        def dmaa(out, in_):          # ACT queue: fsh, idxw loads, uq, out
            nc.scalar.dma_start(out=out, in_=in_)

        # ---------------- constants ----------------
        def cload(name, q=None):
            shape, dt = PARAM_SPECS[name]
            t = consts.tile(list(shape), dt, tag=name, name=f"c_{name}")
            (q or dmas)(t[:], P[name].ap())
            return t

        def dmag(out, in_):
            nc.gpsimd.dma_start(out=out, in_=in_)

        wv_sb = cload('wv_t')
        woff_sb = cload('woff_t')
        boff_sb = cload('boff_p')
        wsz_sb = cload('wsz_t')
        bsz_sb = cload('bsz_p')
        sel8_sb = cload('sel8')
        watt_sb = cload('watt_t')
        batt_sb = cload('batt_r')
        ones_sb = cload('ones1')
        id_sb = cload('ident')
        wout_sb = cload('wout_t')
        obias_sb = cload('obias')
        cen_sb = cload('cen2')

        mem_t = {}
        st = {}

        # ---------------- value conv (bf16 quads, no bias) ----------------
        def conv_sec(b, hgs=(0, 1)):
            qs = {}
            for hg in hgs:
                q = memp.tile([128, L, 4], BF16, tag="quad",
                              name=f"quad{b}{hg}", bufs=2)
                mem_t[(b, hg)] = q
                nc.vector.memset(q[:, L - 65:, :], 0.0)
                qs[hg] = q
            for c in range(4):
                ft = featp.tile([128, 2, 1024], BF16, tag="ft")
                ftq = (dmas, dmaa,
                       lambda o, i: nc.gpsimd.dma_start(out=o, in_=i))[c % 3]
                ftq(ft[:], AP(tensor=P['featb'], offset=b * 2 * L + c * 1024,
                              ap=[[B * 2 * L, 128], [L, 2], [1, 1024]]))
                for j in range(2):
                    n = c * 2 + j
                    for hg in hgs:
                        q = qs[hg]
                        ps = ps_v.tile([128, 512], F32, tag="pv")
                        for kc in range(2):
                            nc.tensor.matmul(ps[:], wv_sb[:, hg, kc, :],
                                             ft[:, kc, j * 512:(j + 1) * 512],
                                             start=(kc == 0), stop=(kc == 1))
                        for dxi, sh in ((0, 0), (1, 1), (2, 64), (3, 65)):
                            if sh == 0:
                                o, i = q[:, n * 512:(n + 1) * 512, 0], ps[:]
                            elif n == 0:
                                o, i = q[:, 0:512 - sh, dxi], ps[:, sh:512]
                            else:
                                o = q[:, n * 512 - sh:(n + 1) * 512 - sh, dxi]
                                i = ps[:]
                            if dxi < 2:
                                nc.scalar.activation(out=o, in_=i,
                                                     func=Act.Copy)
                            else:
                                nc.vector.tensor_copy(out=o, in_=i)

        # ---------------- prep phases (per b) ----------------
        def prep_a(b):
            """fsh load; offset+size convs -> packed offp/szbp [128,2,LSH]."""
            s = st.setdefault(b, {})
            fsh = fshp.tile([128, 2, LSH], F32R, tag="fsh")
            dmaa(fsh[:], P['fsh'].ap()[:, b, :, :])
            s['fsh'] = fsh
            offp = prep.tile([128, 2, LSH], F32, tag="offp")
            szbp = prep.tile([128, 2, LSH], F32, tag="szbp")
            s['offp'], s['szbp'] = offp, szbp
            for xy in range(2):
                ps = ps_p.tile([128, 512], F32, tag="pp", name="psz")
                for kc in range(2):
                    nc.tensor.matmul(ps[0:8, :], wsz_sb[:, xy, kc, :],
                                     fsh[:, kc, :], start=(kc == 0),
                                     stop=(kc == 1))
                szs = gm.tile([8, LSH], F32, tag="szs")
                nc.scalar.activation(out=szs[:], in_=ps[0:8, :],
                                     func=Act.Sigmoid,
                                     bias=bsz_sb[:, xy:xy + 1], scale=1.0)
                nc.vector.tensor_scalar(out=szs[:], in0=szs[:], scalar1=0.75,
                                        scalar2=0.25, op0=Alu.min, op1=Alu.max)
                psb = ps_p.tile([128, 512], F32, tag="pp", name="psb")
                nc.tensor.matmul(psb[:], sel8_sb[:], szs[:],
                                 start=True, stop=True)
                nc.vector.tensor_copy(out=szbp[:, xy, :], in_=psb[:])
                ps2 = ps_p.tile([128, 512], F32, tag="pp", name="po")
                for kc in range(2):
                    nc.tensor.matmul(ps2[:], woff_sb[:, xy, kc, :],
                                     fsh[:, kc, :], start=(kc == 0),
                                     stop=(kc == 1))
                nc.scalar.activation(out=offp[:, xy, :], in_=ps2[:],
                                     func=Act.Sigmoid,
                                     bias=boff_sb[:, xy:xy + 1], scale=1.0)

        def prep_c(b):
            """grid -> floor -> flat y0 idx (packed x|y in one [128,2,LSH])."""
            s = st[b]
            offp, szbp = s['offp'], s['szbp']
            o2 = offp[:].rearrange("p a b -> p (a b)")
            s2 = szbp[:].rearrange("p a b -> p (a b)")
            cf = gm.tile([128, 2, LSH], F32, tag="cf")
            c2 = cf[:].rearrange("p a b -> p (a b)")
            ci = gm.tile([128, 2, LSH], I16, tag="ci")
            i2 = ci[:].rearrange("p a b -> p (a b)")
            msk = gm.tile([128, 2, LSH], F32, tag="msk")
            m2 = msk[:].rearrange("p a b -> p (a b)")
            nc.vector.tensor_scalar(out=o2, in0=o2, scalar1=-0.5,
                                    scalar2=None, op0=Alu.add)
            nc.vector.tensor_tensor(out=o2, in0=o2, in1=s2, op=Alu.mult)
            nc.vector.tensor_tensor(
                out=o2, in0=o2,
                in1=cen_sb[:].rearrange("p a b -> p (a b)"), op=Alu.add)
            nc.vector.tensor_scalar(out=o2, in0=o2, scalar1=float(W - 1),
                                    scalar2=0.0, op0=Alu.min, op1=Alu.max)
            nc.vector.tensor_copy(out=i2, in_=o2)
            nc.vector.tensor_copy(out=c2, in_=i2)
            nc.vector.tensor_tensor(out=m2, in0=c2, in1=o2, op=Alu.is_gt)
            nc.vector.tensor_tensor(out=c2, in0=c2, in1=m2, op=Alu.subtract)
            nc.vector.tensor_tensor(out=o2, in0=o2, in1=c2, op=Alu.subtract)
            # flat y0 = y0f*W + x0f -> reuse szbp x-half as scratch, fi -> ci
            fl = szbp[:, 0, :]
            nc.vector.tensor_scalar(out=fl, in0=cf[:, 1, :],
                                    scalar1=float(W), scalar2=None,
                                    op0=Alu.mult)
            nc.vector.tensor_tensor(out=fl, in0=fl, in1=cf[:, 0, :],
                                    op=Alu.add)
            fi = gm.tile([128, LSH], I16, tag="fi")
            nc.vector.tensor_copy(out=fi[:], in_=fl)
            s['fi'] = fi
            # wx/wy in offp halves; cf/msk slots free for prep_d reuse
            s['cf'], s['msk'] = cf, msk

        def prep_cs(b, k):
            """Wrapped y0 idx stores for l-block k + clean idxw loads +
            DVE-derived y1 idx tiles."""
            s = st[b]
            fi = s['fi']
            for hg in range(2):
                # one store for the whole head-group: partitions (hh,p) merge
                # to a single stride-16 dim in [b][lq][hh][p][lhi]
                dmas(AP(tensor=idxd[(k, hg)], offset=b * 16384,
                        ap=[[16, 64], [1, 16], [1024, 16]]),
                     fi[hg * 64:(hg + 1) * 64, k * 256:(k + 1) * 256])
                ix = idxwp.tile([128, 256], I16, tag=f"ix{hg}{k}",
                                name=f"ix{b}{hg}{k}")
                st[('ix', b, hg, k)] = ix
                for hh in range(4):
                    # 32 contiguous partitions (dup 2 x lq 16) per load; the
                    # dup replication is a stride-0 source dim
                    dmaa(ix[hh * 32:(hh + 1) * 32, :],
                         AP(tensor=idxd[(k, hg)],
                            offset=b * 16384 + hh * 256,
                            ap=[[0, 2], [1024, 16], [1, 256]]))

        def prep_b(b):
            """attn conv (pixel-major) + softmax + transpose -> aT."""
            s = st[b]
            fsh = s['fsh']
            aT = prep.tile([128, LSH], F32, tag="aT")
            s['aT'] = aT
            for lb in range(LSH // 128):
                ps = ps_p.tile([128, 128], F32, tag="pp", name="pa")
                for kc in range(2):
                    nc.tensor.matmul(ps[:], fsh[:, kc, lb * 128:(lb + 1) * 128],
                                     watt_sb[:, kc, :], start=(kc == 0),
                                     stop=False)
                nc.tensor.matmul(ps[:], ones_sb[:], batt_sb[:],
                                 start=False, stop=True)
                ae = gm.tile([128, 8, 16], F32, tag="ae")
                nc.scalar.activation(out=ae[:], in_=ps[:], func=Act.Exp)
                ssum = gm.tile([128, 8, 1], F32, tag="ssum")
                nc.vector.tensor_reduce(out=ssum[:], in_=ae[:],
                                        axis=mybir.AxisListType.X, op=Alu.add)
                nc.vector.reciprocal(out=ssum[:], in_=ssum[:])
                for h in range(NH):
                    nc.vector.tensor_scalar(out=ae[:, h, :], in0=ae[:, h, :],
                                            scalar1=ssum[:, h, :],
                                            scalar2=None, op0=Alu.mult)
                pst = ps_p.tile([128, 128], F32, tag="pp", name="pt")
                nc.tensor.transpose(pst[:], ae[:].rearrange("p a b -> p (a b)"),
                                    id_sb[:])
                nc.scalar.activation(out=aT[:, lb * 128:(lb + 1) * 128],
                                     in_=pst[:], func=Act.Copy)

        def prep_d(b):
            """u = attn * bilinear -> upair [128, tap, l, dx] -> DRAM."""
            s = st[b]
            offp, cf, msk, aT = s['offp'], s['cf'], s['msk'], s['aT']
            wx, wy = offp[:, 0, :], offp[:, 1, :]
            omx, omy = msk[:, 0, :], msk[:, 1, :]
            ay0, ay1 = cf[:, 0, :], cf[:, 1, :]
            nc.vector.tensor_scalar(out=omx, in0=wx, scalar1=-1.0,
                                    scalar2=1.0, op0=Alu.mult, op1=Alu.add)
            nc.vector.tensor_scalar(out=omy, in0=wy, scalar1=-1.0,
                                    scalar2=1.0, op0=Alu.mult, op1=Alu.add)
            nc.vector.tensor_tensor(out=ay0, in0=aT[:], in1=omy, op=Alu.mult)
            nc.vector.tensor_tensor(out=ay1, in0=aT[:], in1=wy, op=Alu.mult)
            uq = gm.tile([128, LSH, 4], BF16, tag="uq")
            for dxi, (yf, xf) in enumerate(((ay0, omx), (ay0, wx),
                                            (ay1, omx), (ay1, wx))):
                nc.vector.tensor_tensor(out=uq[:, :, dxi], in0=yf,
                                        in1=xf, op=Alu.mult)
            dmaa(AP(tensor=ud, offset=b * 128 * 2048,
                    ap=[[2048, 128], [1, 2048]]),
                 uq[:].rearrange("p a b -> p (a b)"))

        # ---------------- gather + combine ----------------
        pso = {}
        cnt = {}
        ubc_rr = [0]

        def gblock(b, hg, k):
            quad = mem_t[(b, hg)]
            idxw = st[('ix', b, hg, k)]
            for oc in range(2):
                if (b, k, oc) not in pso:
                    pso[(b, k, oc)] = ps_o.tile([128, 256], F32,
                                                tag=f"po{k}{oc}",
                                                name=f"po{b}{k}{oc}")
                    cnt[(b, k, oc)] = 0
            g = gathp.tile([128, 4096, 4], BF16, tag="g4")
            # f32-pair view: one gathered "element" = 2 packed bf16 taps, so
            # d=2 f32 instead of d=4 bf16 halves the gather's free-size cost.
            qf = quad[:].rearrange("p a b -> p (a b)").bitcast(F32)
            if stub_gather:
                nc.gpsimd.ap_gather(
                    g[:, 0:16, :].rearrange("p a b -> p (a b)").bitcast(F32),
                    qf, idxw[:, 0:1], channels=128,
                    num_elems=L, d=2, num_idxs=16)
            else:
                nc.gpsimd.ap_gather(
                    g[:].rearrange("p a b -> p (a b)").bitcast(F32),
                    qf, idxw[:], channels=128,
                    num_elems=L, d=2, num_idxs=4096)
            for jh in range(2):       # half u-broadcasts: p in [8jh, 8jh+8)
                ubc = ubcp.tile([128, 8192], BF16, tag="ubc", bufs=1)
                for hh in range(4):
                    # balance broadcast traffic across SP/ACT/Pool DMA queues
                    r = ubc_rr[0] % 3
                    ubc_rr[0] += 1
                    dmaq = (dmas, dmaa,
                            (lambda o, i: nc.gpsimd.dma_start(out=o, in_=i))
                            )[r]
                    dmaq(ubc[hh * 32:(hh + 1) * 32, :],
                         AP(tensor=ud,
                            offset=(b * 128 + (hg * 4 + hh) * 16 + jh * 8)
                            * 2048 + k * 1024,
                            ap=[[0, 32], [2048, 8], [1, 1024]]))
                nc.vector.tensor_tensor(
                    out=g[:, jh * 2048:(jh + 1) * 2048, :].rearrange(
                        "p a b -> p (a b)"),
                    in0=g[:, jh * 2048:(jh + 1) * 2048, :].rearrange(
                        "p a b -> p (a b)"),
                    in1=ubc[:], op=Alu.mult)
            gap = g[:]
            for oc in range(2):
                for p in range(16):
                    for dxi in range(4):
                        rhs = AP(tensor=gap.tensor,
                                 offset=gap.offset + p * 1024 + dxi,
                                 ap=[gap.ap[0], [4, 256]])
                        c = cnt[(b, k, oc)]
                        nc.tensor.matmul(
                            pso[(b, k, oc)][:],
                            wout_sb[:, hg, oc, :], rhs,
                            start=(c == 0), stop=(c == 127))
                        cnt[(b, k, oc)] = c + 1

        def finalize(b, k):
            for oc in range(2):
                o_sb = outp.tile([128, 256], F32, tag="osb")
                nc.scalar.activation(out=o_sb[:], in_=pso[(b, k, oc)][:],
                                     func=Act.Identity,
                                     bias=obias_sb[:, oc:oc + 1], scale=1.0)
                dmaa(AP(tensor=out_e,
                        offset=((b * 2 + oc) * 128) * LSH + k * 256,
                        ap=[[LSH, 128], [1, 256]]), o_sb[:])

        # ---------------- emission schedule ----------------
        prep_a(0)
        prep_c(0)
        conv_sec(0)
        prep_cs(0, 0)
        prep_b(0)
        prep_d(0)
        gblock(0, 0, 0)
        prep_cs(0, 1)
        gblock(0, 0, 1)
        prep_a(1)
        prep_b(1)
        prep_c(1)
        prep_d(1)
        gblock(0, 1, 0)
        finalize(0, 0)
        prep_cs(1, 0)
        gblock(0, 1, 1)
        finalize(0, 1)
        prep_cs(1, 1)
        conv_sec(1, (0,))
        conv_sec(1, (1,))
        gblock(1, 0, 0)
        gblock(1, 0, 1)
        gblock(1, 1, 0)
        finalize(1, 0)
        gblock(1, 1, 1)
        finalize(1, 1)

    nc.compile()
    _GRAPH_CACHE[key] = nc
    return nc


def stage_inputs(inputs, core):
    """Build the per-core in_map (all arrays pre-laid-out for plain DMAs)."""
    bf16 = ml_dtypes.bfloat16
    feat = np.ascontiguousarray(
        np.asarray(inputs['feat_sd'], np.float32).reshape(B, C, L))
    lo = core * LSH
    WvT = np.asarray(inputs['value_proj_w'], np.float32).T.copy()
    WoffT = np.asarray(inputs['anchor_deform_w'], np.float32).T.copy()
    WattT = np.asarray(inputs['anchor_att_w'], np.float32).T.copy()
    WszT = np.asarray(inputs['size_deform_w'], np.float32).T.copy()
    WoutT = np.asarray(inputs['out_proj_w'], np.float32).T.copy()
    boff = np.asarray(inputs['anchor_deform_b'], np.float32)
    bsz = np.asarray(inputs['size_deform_b'], np.float32)
    bv = np.asarray(inputs['value_proj_b'], np.float32)
    bn_s = (np.asarray(inputs['bn_gamma'], np.float32)
            / np.sqrt(np.float32(1.0 + 1e-5)))
    beta = np.asarray(inputs['bn_beta'], np.float32)
    WoutT_sc = WoutT * bn_s[None, :]
    obias = (bv @ WoutT_sc + beta).reshape(2, 128).T
    sel8 = np.zeros((8, 128), np.float32)
    for h in range(8):
        sel8[h, h * 16:(h + 1) * 16] = float(W - 1)
    cols = (np.arange(W) + 0.5) / (W + EPS)
    rows = (np.arange(H) + 0.5) / (H + EPS)
    cx = np.tile(cols, H)[lo:lo + LSH].astype(np.float32)
    cy = np.repeat(rows, W)[lo:lo + LSH].astype(np.float32)
    cen2 = np.stack([np.broadcast_to(cx, (128, LSH)),
                     np.broadcast_to(cy, (128, LSH))], axis=1) * (W - 1.0)
    # woff/wsz packed: xy-interleaved output channels split into x|y planes
    woff = np.stack([WoffT[:, 0::2], WoffT[:, 1::2]],
                    axis=1)                      # [256, 2, 128]
    woff_t = woff.reshape(2, 128, 2, 128).transpose(1, 2, 0, 3)
    wsz = np.stack([WszT[:, 0::2], WszT[:, 1::2]], axis=1)  # [256, 2, 8]
    wsz_t = wsz.reshape(2, 128, 2, 8).transpose(1, 2, 0, 3)
    fr = feat.reshape(B, 2, 128, L)
    m = {
        'featb': np.ascontiguousarray(
            fr.transpose(2, 0, 1, 3)).astype(bf16),
        'fsh': np.ascontiguousarray(
            fr[:, :, :, lo:lo + LSH].transpose(2, 0, 1, 3)),
        'wv_t': np.ascontiguousarray(
            WvT.reshape(2, 128, 2, 128).transpose(1, 2, 0, 3)).astype(bf16),
        'woff_t': np.ascontiguousarray(woff_t),
        'boff_p': np.ascontiguousarray(
            np.stack([boff[0::2], boff[1::2]], axis=1)),
        'wsz_t': np.ascontiguousarray(wsz_t),
        'bsz_p': np.ascontiguousarray(
            np.stack([bsz[0::2], bsz[1::2]], axis=1)),
        'sel8': sel8,
        'watt_t': np.ascontiguousarray(
            WattT.reshape(2, 128, 128).transpose(1, 0, 2)),
        'batt_r': np.asarray(inputs['anchor_att_b'],
                             np.float32).reshape(1, 128),
        'ones1': np.ones((1, 128), np.float32),
        'ident': np.eye(128, dtype=np.float32),
        'wout_t': np.ascontiguousarray(
            WoutT_sc.reshape(2, 128, 2, 128).transpose(1, 0, 2, 3)
        ).astype(bf16),
        'obias': np.ascontiguousarray(obias),
        'cen2': np.ascontiguousarray(cen2),
    }
    return m


def kernel(**inputs):
    nc = build_graph()
    in_maps = [stage_inputs(inputs, i) for i in range(NCORES)]
    res = run_bass_kernel_spmd(nc, in_maps, core_ids=list(range(NCORES)))
    shards = [res.results[i]['out'].reshape(B, C, LSH) for i in range(NCORES)]
    full = np.concatenate(shards, axis=2).reshape(B, C, H, W)
    return full.astype(np.float32)



# revision 29
# speedup vs baseline: 2.0708x; 1.9877x over previous
"""Trainium2 Bass kernel for nn_AnchorDeformAtt (deformable anchor attention).

Sharding: spatial L-shard across 8 cores -- core i handles pixels
l in [512i, 512(i+1)) for BOTH batches and ALL heads. Zero collectives;
the host concatenates per-core output shards.

Design (the gather is ~75% of runtime; everything else hides under it):
  - Memory: quad rows (m[j], m[j+1], m[j+64], m[j+65]) bf16, so ONE
    index per sample point fetches all 4 bilinear taps (d=4 bf16 --
    measured faster on HW than the equivalent d=2 f32-pair view).
  - The GPSIMD engine runs ONLY ap_gathers: Pool-issued DMAs go through
    SWDGE holding the engine and head-of-line-block the gather queue
    (measured ~370us); all DMA issue lives on the SP/ACT queues.
  - Value conv in bf16 (feat staged bf16 host-side); bias folded out
    (softmax x bilinear weights sum to 1 => Wout @ bv is a constant
    output bias, merged with bn_beta host-side; BN scale folded into
    Wout). Prep convs in float32r (tf32 rate).
  - Gather stream per head 16-partition group: i = (p, lhi, lq) with
    row q = lq = l%16, cols (p, lhi). Indices are wrapped at STORE
    time into DRAM [b][lq][hh][p][lhi] (single 3-dim store per (b,hg,k)
    since (hh,p) merges to one stride-16 dim); loads replicate the
    16-row groups with a stride-0 dup dim.
  - f32->i16 index conversion ROUNDS on HW (CoreSim truncates): the
    is_gt/subtract pass after the copy restores floor semantics.
  - u = attn * bilinear weights staged as [b][(h,p)][l][dx4],
    broadcast to each head's 32 channels with 3-dim-AP DMAs (cheap:
    fully hidden under the gather once off the Pool queue).
  - Combine (dve_combine): per (b,hg,k) the gathered [128ch,(p,l,dx)]
    tile is multiplied by u with a permuted write into gu[(l,p,dx)],
    then an in-place bf16 pairwise add tree reduces (p,dx) -- 7 DVE
    ops replace 128 accumulating matmuls; out_proj is 2 matmuls per
    gblock (stride-64 rhs on the reduced column). ~200us faster on HW
    than the PE-combine path and leaves PE nearly idle.
  - Emission schedule: idx path + hg0 quad first so the first gather
    starts as early as possible; prep/convs/combine fill in under the
    gather stream.
"""
from contextlib import ExitStack

import numpy as np
import ml_dtypes

import concourse.bass as bass
import concourse.mybir as mybir
import concourse.tile as tile
from concourse import bacc
from concourse.bass_utils import run_bass_kernel_spmd

NH, NP = 8, 16
B, C, H, W = 2, 256, 64, 64
L = H * W            # 4096
NCORES = 8
LSH = L // NCORES    # 512
LPAD = L + 64        # pairs-memory rows (y1 tap reads idx+64)
EPS = 1e-6
F32 = mybir.dt.float32
BF16 = mybir.dt.bfloat16
I16 = mybir.dt.int16
F32R = mybir.dt.float32r
FP8 = mybir.dt.float8e4

_GRAPH_CACHE = {}

PARAM_SPECS = {
    'featb': ([128, B, 2, L], BF16),
    'fsh': ([128, B, 2, LSH], F32R),
    'wv_t': ([128, 2, 2, 128], BF16),    # K, hg, kc, M
    'woff_t': ([128, 2, 2, 128], F32R),   # K, xy, kc, M
    'boff_p': ([128, 2], F32),           # per-partition bias, xy
    'wsz_t': ([128, 2, 2, 8], F32R),      # K, xy, kc, 8
    'bsz_p': ([8, 2], F32),
    'sel8': ([8, 128], F32),             # sel8[h, h*16+p] = 1
    'watt_t': ([128, 2, 128], F32R),      # K, kc, N
    'batt_r': ([1, 128], F32),
    'ones1': ([1, 128], F32),
    'ident': ([128, 128], F32),
    'wout_t': ([128, 2, 2, 128], BF16),  # K, hg, oc, M (bn-scaled)
    'obias': ([128, 2], F32),            # (Wout_sc @ bv + beta) as [m, oc]
    'cen2': ([128, 2, LSH], F32),        # packed (x|y) centers
}


def build_graph(stub_gather=False, gather_f32=False, ubc_half=False,
                dve_combine=True, stub_ubc=False):
    key = (stub_gather, gather_f32, ubc_half, dve_combine, stub_ubc)
    if key in _GRAPH_CACHE:
        return _GRAPH_CACHE[key]

    nc = bacc.Bacc("TRN2", target_bir_lowering=False, debug=False,
                   num_devices=NCORES)
    dp = nc.declare_dram_parameter
    P = {n: dp(n, s, dt, isOutput=False) for n, (s, dt) in PARAM_SPECS.items()}
    out_e = dp("out", [B, 2, 128, LSH], F32, isOutput=True)

    # wrapped y0 idx staging, one tensor per (k, hg): [b][lq][hh][p][lhi]
    # (lq-major so the (hh,p) partition dims merge to one stride-16 dim)
    idxd = {(k, hg): nc.dram_tensor(f"idxd{k}{hg}", [B, 16, 4, 16, 16], I16)
            for k in range(2) for hg in range(2)}
    # u staging: [b][(h,p) 128][tap 2][l 512][dx 2]
    ud = nc.dram_tensor("ud", [B, 128, 2, LSH, 2], BF16)

    AP = bass.AP
    Act = mybir.ActivationFunctionType
    Alu = mybir.AluOpType

    with tile.TileContext(nc) as tc, ExitStack() as ctx:
        consts = ctx.enter_context(tc.tile_pool(name="consts", bufs=1))
        featp = ctx.enter_context(tc.tile_pool(name="featp", bufs=2))
        fshp = ctx.enter_context(tc.tile_pool(name="fshp", bufs=1))
        memp = ctx.enter_context(tc.tile_pool(name="memp", bufs=1))
        prep = ctx.enter_context(tc.tile_pool(name="prep", bufs=1))
        gm = ctx.enter_context(tc.tile_pool(name="gm", bufs=1))
        idxwp = ctx.enter_context(tc.tile_pool(name="idxwp", bufs=2))
        ubcp = ctx.enter_context(tc.tile_pool(name="ubcp", bufs=2))
        gathp = ctx.enter_context(tc.tile_pool(name="gathp", bufs=2))
        gup = ctx.enter_context(tc.tile_pool(name="gup", bufs=1))
        outp = ctx.enter_context(tc.tile_pool(name="outp", bufs=2))
        ps_v = ctx.enter_context(tc.tile_pool(name="ps_v", bufs=2, space="PSUM"))
        ps_p = ctx.enter_context(tc.tile_pool(name="ps_p", bufs=2, space="PSUM"))
        ps_o = ctx.enter_context(tc.tile_pool(name="ps_o", bufs=1, space="PSUM"))

        def dmas(out, in_):          # SP queue: consts, ft, idx stores, ubc
            nc.sync.dma_start(out=out, in_=in_)

        def dmaa(out, in_):          # ACT queue: fsh, idxw loads, uq, out
            nc.scalar.dma_start(out=out, in_=in_)

        # ---------------- constants ----------------
        def cload(name, q=None):
            shape, dt = PARAM_SPECS[name]
            t = consts.tile(list(shape), dt, tag=name, name=f"c_{name}")
            (q or dmas)(t[:], P[name].ap())
            return t

        def dmag(out, in_):
            nc.gpsimd.dma_start(out=out, in_=in_)

        wv_sb = cload('wv_t')
        woff_sb = cload('woff_t')
        boff_sb = cload('boff_p')
        wsz_sb = cload('wsz_t')
        bsz_sb = cload('bsz_p')
        sel8_sb = cload('sel8')
        watt_sb = cload('watt_t')
        batt_sb = cload('batt_r')
        ones_sb = cload('ones1')
        id_sb = cload('ident')
        wout_sb = cload('wout_t')
        obias_sb = cload('obias')
        cen_sb = cload('cen2')

        mem_t = {}
        st = {}

        # ---------------- value conv (bf16 quads, no bias) ----------------
        def conv_sec(b, hgs=(0, 1)):
            qs = {}
            for hg in hgs:
                q = memp.tile([128, L, 4], BF16, tag="quad",
                              name=f"quad{b}{hg}", bufs=2)
                mem_t[(b, hg)] = q
                nc.vector.memset(q[:, L - 65:, :], 0.0)
                qs[hg] = q
            for c in range(4):
                ft = featp.tile([128, 2, 1024], BF16, tag="ft")
                ftq = (dmas, dmaa)[c % 2]
                ftq(ft[:], AP(tensor=P['featb'], offset=b * 2 * L + c * 1024,
                              ap=[[B * 2 * L, 128], [L, 2], [1, 1024]]))
                for j in range(2):
                    n = c * 2 + j
                    for hg in hgs:
                        q = qs[hg]
                        ps = ps_v.tile([128, 512], F32, tag="pv")
                        for kc in range(2):
                            nc.tensor.matmul(ps[:], wv_sb[:, hg, kc, :],
                                             ft[:, kc, j * 512:(j + 1) * 512],
                                             start=(kc == 0), stop=(kc == 1))
                        for dxi, sh in ((0, 0), (1, 1), (2, 64), (3, 65)):
                            if sh == 0:
                                o, i = q[:, n * 512:(n + 1) * 512, 0], ps[:]
                            elif n == 0:
                                o, i = q[:, 0:512 - sh, dxi], ps[:, sh:512]
                            else:
                                o = q[:, n * 512 - sh:(n + 1) * 512 - sh, dxi]
                                i = ps[:]
                            if dxi < 2:
                                nc.scalar.activation(out=o, in_=i,
                                                     func=Act.Copy)
                            else:
                                nc.vector.tensor_copy(out=o, in_=i)

        # ---------------- prep phases (per b) ----------------
        def prep_a(b):
            """fsh load; offset+size convs -> packed offp/szbp [128,2,LSH]."""
            s = st.setdefault(b, {})
            fsh = fshp.tile([128, 2, LSH], F32R, tag="fsh")
            dmaa(fsh[:], P['fsh'].ap()[:, b, :, :])
            s['fsh'] = fsh
            offp = prep.tile([128, 2, LSH], F32, tag="offp")
            szbp = prep.tile([128, 2, LSH], F32, tag="szbp")
            s['offp'], s['szbp'] = offp, szbp
            for xy in range(2):
                ps = ps_p.tile([128, 512], F32, tag="pp", name="psz")
                for kc in range(2):
                    nc.tensor.matmul(ps[0:8, :], wsz_sb[:, xy, kc, :],
                                     fsh[:, kc, :], start=(kc == 0),
                                     stop=(kc == 1))
                szs = gm.tile([8, LSH], F32, tag="szs")
                nc.scalar.activation(out=szs[:], in_=ps[0:8, :],
                                     func=Act.Sigmoid,
                                     bias=bsz_sb[:, xy:xy + 1], scale=1.0)
                nc.vector.tensor_scalar(out=szs[:], in0=szs[:], scalar1=0.75,
                                        scalar2=0.25, op0=Alu.min, op1=Alu.max)
                psb = ps_p.tile([128, 512], F32, tag="pp", name="psb")
                nc.tensor.matmul(psb[:], sel8_sb[:], szs[:],
                                 start=True, stop=True)
                nc.vector.tensor_copy(out=szbp[:, xy, :], in_=psb[:])
                ps2 = ps_p.tile([128, 512], F32, tag="pp", name="po")
                for kc in range(2):
                    nc.tensor.matmul(ps2[:], woff_sb[:, xy, kc, :],
                                     fsh[:, kc, :], start=(kc == 0),
                                     stop=(kc == 1))
                nc.scalar.activation(out=offp[:, xy, :], in_=ps2[:],
                                     func=Act.Sigmoid,
                                     bias=boff_sb[:, xy:xy + 1], scale=1.0)

        def prep_c(b):
            """grid -> floor -> flat y0 idx (packed x|y in one [128,2,LSH])."""
            s = st[b]
            offp, szbp = s['offp'], s['szbp']
            o2 = offp[:].rearrange("p a b -> p (a b)")
            s2 = szbp[:].rearrange("p a b -> p (a b)")
            cf = gm.tile([128, 2, LSH], F32, tag="cf")
            c2 = cf[:].rearrange("p a b -> p (a b)")
            ci = gm.tile([128, 2, LSH], I16, tag="ci")
            i2 = ci[:].rearrange("p a b -> p (a b)")
            msk = gm.tile([128, 2, LSH], F32, tag="msk")
            m2 = msk[:].rearrange("p a b -> p (a b)")
            nc.vector.tensor_scalar(out=o2, in0=o2, scalar1=-0.5,
                                    scalar2=None, op0=Alu.add)
            nc.vector.tensor_tensor(out=o2, in0=o2, in1=s2, op=Alu.mult)
            nc.vector.tensor_tensor(
                out=o2, in0=o2,
                in1=cen_sb[:].rearrange("p a b -> p (a b)"), op=Alu.add)
            nc.vector.tensor_scalar(out=o2, in0=o2, scalar1=float(W - 1),
                                    scalar2=0.0, op0=Alu.min, op1=Alu.max)
            # f32->i16 conversion rounds to nearest on HW: compare-and-
            # subtract corrects round-up cases back to floor
            nc.vector.tensor_copy(out=i2, in_=o2)
            nc.vector.tensor_copy(out=c2, in_=i2)
            nc.vector.tensor_tensor(out=m2, in0=c2, in1=o2, op=Alu.is_gt)
            nc.vector.tensor_tensor(out=c2, in0=c2, in1=m2, op=Alu.subtract)
            nc.vector.tensor_tensor(out=o2, in0=o2, in1=c2, op=Alu.subtract)
            # flat y0 = y0f*W + x0f -> reuse szbp x-half as scratch, fi -> ci
            fl = szbp[:, 0, :]
            nc.vector.tensor_scalar(out=fl, in0=cf[:, 1, :],
                                    scalar1=float(W), scalar2=None,
                                    op0=Alu.mult)
            nc.vector.tensor_tensor(out=fl, in0=fl, in1=cf[:, 0, :],
                                    op=Alu.add)
            fi = gm.tile([128, LSH], I16, tag="fi")
            nc.vector.tensor_copy(out=fi[:], in_=fl)
            s['fi'] = fi
            # wx/wy in offp halves; cf/msk slots free for prep_d reuse
            s['cf'], s['msk'] = cf, msk

        def prep_cs(b, k):
            """Wrapped y0 idx stores for l-block k + clean idxw loads +
            DVE-derived y1 idx tiles."""
            s = st[b]
            fi = s['fi']
            for hg in range(2):
                # one store for the whole head-group: partitions (hh,p) merge
                # to a single stride-16 dim in [b][lq][hh][p][lhi]
                dmas(AP(tensor=idxd[(k, hg)], offset=b * 16384,
                        ap=[[16, 64], [1, 16], [1024, 16]]),
                     fi[hg * 64:(hg + 1) * 64, k * 256:(k + 1) * 256])
                ix = idxwp.tile([128, 256], I16, tag=f"ix{hg}{k}",
                                name=f"ix{b}{hg}{k}")
                st[('ix', b, hg, k)] = ix
                for hh in range(4):
                    # 32 contiguous partitions (dup 2 x lq 16) per load; the
                    # dup replication is a stride-0 source dim
                    dmaa(ix[hh * 32:(hh + 1) * 32, :],
                         AP(tensor=idxd[(k, hg)],
                            offset=b * 16384 + hh * 256,
                            ap=[[0, 2], [1024, 16], [1, 256]]))

        def prep_b(b):
            """attn conv (pixel-major) + softmax + transpose -> aT."""
            s = st[b]
            fsh = s['fsh']
            aT = prep.tile([128, LSH], F32, tag="aT")
            s['aT'] = aT
            for lb in range(LSH // 128):
                ps = ps_p.tile([128, 128], F32, tag="pp", name="pa")
                for kc in range(2):
                    nc.tensor.matmul(ps[:], fsh[:, kc, lb * 128:(lb + 1) * 128],
                                     watt_sb[:, kc, :], start=(kc == 0),
                                     stop=False)
                nc.tensor.matmul(ps[:], ones_sb[:], batt_sb[:],
                                 start=False, stop=True)
                ae = gm.tile([128, 8, 16], F32, tag="ae")
                nc.scalar.activation(out=ae[:], in_=ps[:], func=Act.Exp)
                ssum = gm.tile([128, 8, 1], F32, tag="ssum")
                nc.vector.tensor_reduce(out=ssum[:], in_=ae[:],
                                        axis=mybir.AxisListType.X, op=Alu.add)
                nc.vector.reciprocal(out=ssum[:], in_=ssum[:])
                ssa = ssum[:]
                nc.vector.tensor_tensor(
                    out=ae[:], in0=ae[:],
                    in1=AP(tensor=ssa.tensor, offset=ssa.offset,
                           ap=[ssa.ap[0], [1, 8], [0, 16]]),
                    op=Alu.mult)
                pst = ps_p.tile([128, 128], F32, tag="pp", name="pt")
                nc.tensor.transpose(pst[:], ae[:].rearrange("p a b -> p (a b)"),
                                    id_sb[:])
                nc.scalar.activation(out=aT[:, lb * 128:(lb + 1) * 128],
                                     in_=pst[:], func=Act.Copy)

        def prep_d(b):
            """u = attn * bilinear -> upair [128, tap, l, dx] -> DRAM."""
            s = st[b]
            offp, cf, msk, aT = s['offp'], s['cf'], s['msk'], s['aT']
            wx, wy = offp[:, 0, :], offp[:, 1, :]
            omx, omy = msk[:, 0, :], msk[:, 1, :]
            ay0, ay1 = cf[:, 0, :], cf[:, 1, :]
            nc.vector.tensor_scalar(out=omx, in0=wx, scalar1=-1.0,
                                    scalar2=1.0, op0=Alu.mult, op1=Alu.add)
            nc.vector.tensor_scalar(out=omy, in0=wy, scalar1=-1.0,
                                    scalar2=1.0, op0=Alu.mult, op1=Alu.add)
            nc.vector.tensor_tensor(out=ay0, in0=aT[:], in1=omy, op=Alu.mult)
            nc.vector.tensor_tensor(out=ay1, in0=aT[:], in1=wy, op=Alu.mult)
            uq = gm.tile([128, LSH, 4], BF16, tag="uq")
            for dxi, (yf, xf) in enumerate(((ay0, omx), (ay0, wx),
                                            (ay1, omx), (ay1, wx))):
                nc.vector.tensor_tensor(out=uq[:, :, dxi], in0=yf,
                                        in1=xf, op=Alu.mult)
            dmaa(AP(tensor=ud, offset=b * 128 * 2048,
                    ap=[[2048, 128], [1, 2048]]),
                 uq[:].rearrange("p a b -> p (a b)"))

        # ---------------- gather + combine ----------------
        pso = {}
        cnt = {}
        ubc_rr = [0]

        def gblock(b, hg, k):
            quad = mem_t[(b, hg)]
            idxw = st[('ix', b, hg, k)]
            for oc in range(2):
                if (b, k, oc) not in pso:
                    pso[(b, k, oc)] = ps_o.tile([128, 256], F32,
                                                tag=f"po{k}{oc}",
                                                name=f"po{b}{k}{oc}")
                    cnt[(b, k, oc)] = 0
            if dve_combine:
                # p-half d4 gathers -> permuted multiply into gu (l,p,dx) ->
                # in-place bf16 add tree over (p,dx) -> 2 out_proj matmuls
                gu = gup.tile([128, 256, 16, 4], BF16, tag="gu", bufs=1)
                gua = gu[:]
                ghs = []
                for jh in range(2):
                    gh = gathp.tile([128, 2048, 4], BF16, tag="g4h")
                    ghs.append(gh)
                    if stub_gather:
                        nc.gpsimd.ap_gather(
                            gh[:, 0:16, :],
                            quad[:].rearrange("p a b -> p (a b)"),
                            idxw[:, 0:1], channels=128,
                            num_elems=L, d=4, num_idxs=16)
                    else:
                        nc.gpsimd.ap_gather(
                            gh[:], quad[:].rearrange("p a b -> p (a b)"),
                            idxw[:, jh * 128:(jh + 1) * 128],
                            channels=128, num_elems=L, d=4, num_idxs=2048)
                for jq in range(4):
                    ubc = ubcp.tile([128, 4096], BF16, tag="ubc")
                    for hh in range(4):
                        r = ubc_rr[0] % 2
                        ubc_rr[0] += 1
                        dmaq = (dmas, dmaa)[r]
                        if stub_ubc and hh > 0:
                            continue
                        dmaq(ubc[hh * 32:(hh + 1) * 32, :],
                             AP(tensor=ud,
                                offset=(b * 128 + (hg * 4 + hh) * 16
                                        + jq * 4) * 2048 + k * 1024,
                                ap=[[0, 32], [2048, 4], [1, 1024]]))
                    gh = ghs[jq // 2]
                    sl = gh[:, (jq % 2) * 1024:(jq % 2 + 1) * 1024, :]
                    nc.vector.tensor_tensor(
                        out=AP(tensor=gua.tensor,
                               offset=gua.offset + jq * 16,
                               ap=[gua.ap[0], [4, 4], [64, 256], [1, 4]]),
                        in0=sl.rearrange("p a b -> p (a b)"),
                        in1=ubc[:], op=Alu.mult)

                def tadd(o, i1):
                    nc.vector.tensor_tensor(out=o, in0=o, in1=i1, op=Alu.add)
                tadd(gu[:, :, :, 0:2], gu[:, :, :, 2:4])
                tadd(gu[:, :, 0:8, 0:2], gu[:, :, 8:16, 0:2])
                tadd(gu[:, :, 0:4, 0:2], gu[:, :, 4:8, 0:2])
                tadd(gu[:, :, 0:2, 0:2], gu[:, :, 2:4, 0:2])
                tadd(gu[:, :, 0:1, 0:2], gu[:, :, 1:2, 0:2])
                tadd(gu[:, :, 0:1, 0:1], gu[:, :, 0:1, 1:2])
                rhs = AP(tensor=gua.tensor, offset=gua.offset,
                         ap=[gua.ap[0], [64, 256]])
                for oc in range(2):
                    c = cnt[(b, k, oc)]
                    nc.tensor.matmul(pso[(b, k, oc)][:],
                                     wout_sb[:, hg, oc, :], rhs,
                                     start=(c == 0), stop=(c == 1))
                    cnt[(b, k, oc)] = c + 1
                return
            g = gathp.tile([128, 4096, 4], BF16, tag="g4")
            if gather_f32:
                # f32-pair view: one gathered "element" = 2 packed bf16 taps,
                # so d=2 f32 halves the gather's free-size cost.
                qf = quad[:].rearrange("p a b -> p (a b)").bitcast(F32)
                if stub_gather:
                    nc.gpsimd.ap_gather(
                        g[:, 0:16, :].rearrange(
                            "p a b -> p (a b)").bitcast(F32),
                        qf, idxw[:, 0:1], channels=128,
                        num_elems=L, d=2, num_idxs=16)
                else:
                    nc.gpsimd.ap_gather(
                        g[:].rearrange("p a b -> p (a b)").bitcast(F32),
                        qf, idxw[:], channels=128,
                        num_elems=L, d=2, num_idxs=4096)
            elif stub_gather:
                nc.gpsimd.ap_gather(
                    g[:, 0:16, :], quad[:].rearrange("p a b -> p (a b)"),
                    idxw[:, 0:1], channels=128,
                    num_elems=L, d=4, num_idxs=16)
            else:
                nc.gpsimd.ap_gather(
                    g[:], quad[:].rearrange("p a b -> p (a b)"),
                    idxw[:], channels=128,
                    num_elems=L, d=4, num_idxs=4096)
            nj, pq = (2, 8) if ubc_half else (4, 4)
            for jq in range(nj):      # u-broadcasts: p in [pq*jq, pq*(jq+1))
                ubc = ubcp.tile([128, 1024 * pq], BF16, tag="ubc",
                                bufs=1 if ubc_half else 2)
                for hh in range(4):
                    # balance broadcast traffic across SP/ACT/Pool DMA queues
                    r = ubc_rr[0] % 2
                    ubc_rr[0] += 1
                    dmaq = (dmas, dmaa)[r]
                    if stub_ubc and hh > 0:
                        continue
                    dmaq(ubc[hh * 32:(hh + 1) * 32, :],
                         AP(tensor=ud,
                            offset=(b * 128 + (hg * 4 + hh) * 16 + jq * pq)
                            * 2048 + k * 1024,
                            ap=[[0, 32], [2048, pq], [1, 1024]]))
                sl = g[:, jq * pq * 256:(jq + 1) * pq * 256, :].rearrange(
                    "p a b -> p (a b)")
                nc.vector.tensor_tensor(out=sl, in0=sl, in1=ubc[:],
                                        op=Alu.mult)
            gap = g[:]
            for oc in range(2):
                for p in range(16):
                    for dxi in range(4):
                        rhs = AP(tensor=gap.tensor,
                                 offset=gap.offset + p * 1024 + dxi,
                                 ap=[gap.ap[0], [4, 256]])
                        c = cnt[(b, k, oc)]
                        nc.tensor.matmul(
                            pso[(b, k, oc)][:],
                            wout_sb[:, hg, oc, :], rhs,
                            start=(c == 0), stop=(c == 127))
                        cnt[(b, k, oc)] = c + 1

        def finalize(b, k):
            for oc in range(2):
                o_sb = outp.tile([128, 256], F32, tag="osb")
                nc.scalar.activation(out=o_sb[:], in_=pso[(b, k, oc)][:],
                                     func=Act.Identity,
                                     bias=obias_sb[:, oc:oc + 1], scale=1.0)
                dmaa(AP(tensor=out_e,
                        offset=((b * 2 + oc) * 128) * LSH + k * 256,
                        ap=[[LSH, 128], [1, 256]]), o_sb[:])

        # ---------------- emission schedule ----------------
        # idx path + hg0 quad first so the Pool engine's first gather can
        # start as early as possible; everything else fills in under the
        # gather stream.
        prep_a(0)
        prep_c(0)
        prep_cs(0, 0)
        conv_sec(0, (0,))
        prep_b(0)
        prep_d(0)
        prep_cs(0, 1)
        gblock(0, 0, 0)
        conv_sec(0, (1,))
        gblock(0, 0, 1)
        prep_a(1)
        prep_b(1)
        prep_c(1)
        prep_d(1)
        gblock(0, 1, 0)
        finalize(0, 0)
        prep_cs(1, 0)
        gblock(0, 1, 1)
        finalize(0, 1)
        prep_cs(1, 1)
        conv_sec(1, (0,))
        conv_sec(1, (1,))
        gblock(1, 0, 0)
        gblock(1, 0, 1)
        gblock(1, 1, 0)
        finalize(1, 0)
        gblock(1, 1, 1)
        finalize(1, 1)

    nc.compile()
    _GRAPH_CACHE[key] = nc
    return nc


def stage_inputs(inputs, core):
    """Build the per-core in_map (all arrays pre-laid-out for plain DMAs)."""
    bf16 = ml_dtypes.bfloat16
    feat = np.ascontiguousarray(
        np.asarray(inputs['feat_sd'], np.float32).reshape(B, C, L))
    lo = core * LSH
    WvT = np.asarray(inputs['value_proj_w'], np.float32).T.copy()
    WoffT = np.asarray(inputs['anchor_deform_w'], np.float32).T.copy()
    WattT = np.asarray(inputs['anchor_att_w'], np.float32).T.copy()
    WszT = np.asarray(inputs['size_deform_w'], np.float32).T.copy()
    WoutT = np.asarray(inputs['out_proj_w'], np.float32).T.copy()
    boff = np.asarray(inputs['anchor_deform_b'], np.float32)
    bsz = np.asarray(inputs['size_deform_b'], np.float32)
    bv = np.asarray(inputs['value_proj_b'], np.float32)
    bn_s = (np.asarray(inputs['bn_gamma'], np.float32)
            / np.sqrt(np.float32(1.0 + 1e-5)))
    beta = np.asarray(inputs['bn_beta'], np.float32)
    WoutT_sc = WoutT * bn_s[None, :]
    obias = (bv @ WoutT_sc + beta).reshape(2, 128).T
    sel8 = np.zeros((8, 128), np.float32)
    for h in range(8):
        sel8[h, h * 16:(h + 1) * 16] = float(W - 1)
    cols = (np.arange(W) + 0.5) / (W + EPS)
    rows = (np.arange(H) + 0.5) / (H + EPS)
    cx = np.tile(cols, H)[lo:lo + LSH].astype(np.float32)
    cy = np.repeat(rows, W)[lo:lo + LSH].astype(np.float32)
    cen2 = np.stack([np.broadcast_to(cx, (128, LSH)),
                     np.broadcast_to(cy, (128, LSH))], axis=1) * (W - 1.0)
    # woff/wsz packed: xy-interleaved output channels split into x|y planes
    woff = np.stack([WoffT[:, 0::2], WoffT[:, 1::2]],
                    axis=1)                      # [256, 2, 128]
    woff_t = woff.reshape(2, 128, 2, 128).transpose(1, 2, 0, 3)
    wsz = np.stack([WszT[:, 0::2], WszT[:, 1::2]], axis=1)  # [256, 2, 8]
    wsz_t = wsz.reshape(2, 128, 2, 8).transpose(1, 2, 0, 3)
    fr = feat.reshape(B, 2, 128, L)
    m = {
        'featb': np.ascontiguousarray(
            fr.transpose(2, 0, 1, 3)).astype(bf16),
        'fsh': np.ascontiguousarray(
            fr[:, :, :, lo:lo + LSH].transpose(2, 0, 1, 3)),
        'wv_t': np.ascontiguousarray(
            WvT.reshape(2, 128, 2, 128).transpose(1, 2, 0, 3)).astype(bf16),
        'woff_t': np.ascontiguousarray(woff_t),
        'boff_p': np.ascontiguousarray(
            np.stack([boff[0::2], boff[1::2]], axis=1)),
        'wsz_t': np.ascontiguousarray(wsz_t),
        'bsz_p': np.ascontiguousarray(
            np.stack([bsz[0::2], bsz[1::2]], axis=1)),
        'sel8': sel8,
        'watt_t': np.ascontiguousarray(
            WattT.reshape(2, 128, 128).transpose(1, 0, 2)),
        'batt_r': np.asarray(inputs['anchor_att_b'],
                             np.float32).reshape(1, 128),
        'ones1': np.ones((1, 128), np.float32),
        'ident': np.eye(128, dtype=np.float32),
        'wout_t': np.ascontiguousarray(
            WoutT_sc.reshape(2, 128, 2, 128).transpose(1, 0, 2, 3)
        ).astype(bf16),
        'obias': np.ascontiguousarray(obias),
        'cen2': np.ascontiguousarray(cen2),
    }
    return m


def kernel(**inputs):
    nc = build_graph()
    in_maps = [stage_inputs(inputs, i) for i in range(NCORES)]
    res = run_bass_kernel_spmd(nc, in_maps, core_ids=list(range(NCORES)))
    shards = [res.results[i]['out'].reshape(B, C, LSH) for i in range(NCORES)]
    full = np.concatenate(shards, axis=2).reshape(B, C, H, W)
    return full.astype(np.float32)



# revision 36
# speedup vs baseline: 2.1562x; 1.0413x over previous
"""Trainium2 Bass kernel for nn_AnchorDeformAtt (deformable anchor attention).

Sharding: spatial L-shard across 8 cores -- core i handles pixels
l in [512i, 512(i+1)) for BOTH batches and ALL heads. Zero collectives;
the host concatenates per-core output shards.

Design (the gather is ~75% of runtime; everything else hides under it):
  - Memory: quad rows (m[j], m[j+1], m[j+64], m[j+65]) bf16, so ONE
    index per sample point fetches all 4 bilinear taps (d=4 bf16 --
    measured faster on HW than the equivalent d=2 f32-pair view).
  - The GPSIMD engine runs ONLY ap_gathers: Pool-issued DMAs go through
    SWDGE holding the engine and head-of-line-block the gather queue
    (measured ~370us); all DMA issue lives on the SP/ACT queues.
  - Value conv in bf16 (feat staged bf16 host-side); bias folded out
    (softmax x bilinear weights sum to 1 => Wout @ bv is a constant
    output bias, merged with bn_beta host-side; BN scale folded into
    Wout). Prep convs in float32r (tf32 rate).
  - Gather stream per head 16-partition group: i = (p, lhi, lq) with
    row q = lq = l%16, cols (p, lhi). Indices are wrapped at STORE
    time into DRAM [b][lq][hh][p][lhi] (single 3-dim store per (b,hg,k)
    since (hh,p) merges to one stride-16 dim); loads replicate the
    16-row groups with a stride-0 dup dim.
  - f32->i16 index conversion ROUNDS on HW (CoreSim truncates): the
    is_gt/subtract pass after the copy restores floor semantics.
  - u = attn * bilinear weights staged as [b][(h,p)][l][dx4],
    broadcast to each head's 32 channels with 3-dim-AP DMAs (cheap:
    fully hidden under the gather once off the Pool queue).
  - Combine (dve_combine): per (b,hg,k) the gathered [128ch,(p,l,dx)]
    tile is multiplied by u with a permuted write into gu[(l,p,dx)],
    then an in-place bf16 pairwise add tree reduces (p,dx) -- 7 DVE
    ops replace 128 accumulating matmuls; out_proj is 2 matmuls per
    gblock (stride-64 rhs on the reduced column). ~200us faster on HW
    than the PE-combine path and leaves PE nearly idle.
  - Emission schedule: idx path + hg0 quad first so the first gather
    starts as early as possible; conv_sec(1,(0,)) is emitted before
    gblock(0,1,0) so b1's quad copies queue on DVE ahead of the
    gather-dependent mults (removes a ~40us stall at the b transition).
    conv_sec(1,(1,)) must stay after gblock(0,1,1): its memset waits on
    quad01's buffer, whose release needs those mults (DVE deadlock
    otherwise).
  - Measured dead ends (same-process A/B on HW): d=2 f32-pair gather
    (+100us despite half the elements), coarser ubc tiles with bufs=1
    (+200us), one 4096-idx gather per block with gathp bufs=1 (+220us:
    the mult-drain serialization outweighs the ~13us/call fixed cost),
    ubc issue pinned to one queue, and consts alternated onto ACT
    (both regress the fill in the timeline model).
"""
from contextlib import ExitStack

import numpy as np
import ml_dtypes

import concourse.bass as bass
import concourse.mybir as mybir
import concourse.tile as tile
from concourse import bacc
from concourse.bass_utils import run_bass_kernel_spmd

NH, NP = 8, 16
B, C, H, W = 2, 256, 64, 64
L = H * W            # 4096
NCORES = 8
LSH = L // NCORES    # 512
LPAD = L + 64        # pairs-memory rows (y1 tap reads idx+64)
EPS = 1e-6
F32 = mybir.dt.float32
BF16 = mybir.dt.bfloat16
I16 = mybir.dt.int16
F32R = mybir.dt.float32r
FP8 = mybir.dt.float8e4

_GRAPH_CACHE = {}

PARAM_SPECS = {
    'featb': ([128, B, 2, L], BF16),
    'fsh': ([128, B, 2, LSH], F32R),
    'wv_t': ([128, 2, 2, 128], BF16),    # K, hg, kc, M
    'woff_t': ([128, 2, 2, 128], F32R),   # K, xy, kc, M
    'boff_p': ([128, 2], F32),           # per-partition bias, xy
    'wsz_t': ([128, 2, 2, 8], F32R),      # K, xy, kc, 8
    'bsz_p': ([8, 2], F32),
    'sel8': ([8, 128], F32),             # sel8[h, h*16+p] = 1
    'watt_t': ([128, 2, 128], F32R),      # K, kc, N
    'batt_r': ([1, 128], F32),
    'ones1': ([1, 128], F32),
    'ident': ([128, 128], F32),
    'wout_t': ([128, 2, 2, 128], BF16),  # K, hg, oc, M (bn-scaled)
    'obias': ([128, 2], F32),            # (Wout_sc @ bv + beta) as [m, oc]
    'cen2': ([128, 2, LSH], F32),        # packed (x|y) centers
}


def build_graph(stub_gather=False, gather_f32=False, ubc_half=False,
                dve_combine=True, stub_ubc=False, gather8=False,
                qsplit=False):
    key = (stub_gather, gather_f32, ubc_half, dve_combine, stub_ubc, gather8,
           qsplit)
    if key in _GRAPH_CACHE:
        return _GRAPH_CACHE[key]

    nc = bacc.Bacc("TRN2", target_bir_lowering=False, debug=False,
                   num_devices=NCORES)
    dp = nc.declare_dram_parameter
    P = {n: dp(n, s, dt, isOutput=False) for n, (s, dt) in PARAM_SPECS.items()}
    out_e = dp("out", [B, 2, 128, LSH], F32, isOutput=True)

    # wrapped y0 idx staging, one tensor per (k, hg): [b][lq][hh][p][lhi]
    # (lq-major so the (hh,p) partition dims merge to one stride-16 dim)
    idxd = {(k, hg): nc.dram_tensor(f"idxd{k}{hg}", [B, 16, 4, 16, 16], I16)
            for k in range(2) for hg in range(2)}
    # u staging: [b][(h,p) 128][tap 2][l 512][dx 2]
    ud = nc.dram_tensor("ud", [B, 128, 2, LSH, 2], BF16)

    AP = bass.AP
    Act = mybir.ActivationFunctionType
    Alu = mybir.AluOpType

    with tile.TileContext(nc) as tc, ExitStack() as ctx:
        consts = ctx.enter_context(tc.tile_pool(name="consts", bufs=1))
        featp = ctx.enter_context(tc.tile_pool(name="featp", bufs=2))
        fshp = ctx.enter_context(tc.tile_pool(name="fshp", bufs=1))
        memp = ctx.enter_context(tc.tile_pool(name="memp", bufs=1))
        prep = ctx.enter_context(tc.tile_pool(name="prep", bufs=1))
        gm = ctx.enter_context(tc.tile_pool(name="gm", bufs=1))
        idxwp = ctx.enter_context(tc.tile_pool(name="idxwp", bufs=2))
        ubcp = ctx.enter_context(tc.tile_pool(name="ubcp", bufs=2))
        gathp = ctx.enter_context(tc.tile_pool(name="gathp", bufs=2))
        gup = ctx.enter_context(tc.tile_pool(name="gup", bufs=1))
        outp = ctx.enter_context(tc.tile_pool(name="outp", bufs=2))
        ps_v = ctx.enter_context(tc.tile_pool(name="ps_v", bufs=2, space="PSUM"))
        ps_p = ctx.enter_context(tc.tile_pool(name="ps_p", bufs=2, space="PSUM"))
        ps_o = ctx.enter_context(tc.tile_pool(name="ps_o", bufs=1, space="PSUM"))

        def dmas(out, in_):          # SP queue: consts, ft, idx stores, ubc
            nc.sync.dma_start(out=out, in_=in_)

        def dmaa(out, in_):          # ACT queue: fsh, idxw loads, uq, out
            nc.scalar.dma_start(out=out, in_=in_)

        # ---------------- constants ----------------
        crr = [0]

        def cload(name, q=None):
            shape, dt = PARAM_SPECS[name]
            t = consts.tile(list(shape), dt, tag=name, name=f"c_{name}")
            if q is None:
                q = (dmas, dmaa)[crr[0] % 2] if qsplit else dmas
                crr[0] += 1
            q(t[:], P[name].ap())
            return t

        def dmag(out, in_):
            nc.gpsimd.dma_start(out=out, in_=in_)

        wv_sb = cload('wv_t')
        woff_sb = cload('woff_t')
        boff_sb = cload('boff_p')
        wsz_sb = cload('wsz_t')
        bsz_sb = cload('bsz_p')
        sel8_sb = cload('sel8')
        watt_sb = cload('watt_t')
        batt_sb = cload('batt_r')
        ones_sb = cload('ones1')
        id_sb = cload('ident')
        wout_sb = cload('wout_t')
        obias_sb = cload('obias')
        cen_sb = cload('cen2')

        mem_t = {}
        st = {}

        # ---------------- value conv (bf16 quads, no bias) ----------------
        def conv_sec(b, hgs=(0, 1)):
            qs = {}
            for hg in hgs:
                q = memp.tile([128, L, 4], BF16, tag="quad",
                              name=f"quad{b}{hg}", bufs=2)
                mem_t[(b, hg)] = q
                nc.vector.memset(q[:, L - 65:, :], 0.0)
                qs[hg] = q
            for c in range(4):
                ft = featp.tile([128, 2, 1024], BF16, tag="ft")
                ftq = (dmas, dmaa)[c % 2]
                ftq(ft[:], AP(tensor=P['featb'], offset=b * 2 * L + c * 1024,
                              ap=[[B * 2 * L, 128], [L, 2], [1, 1024]]))
                for j in range(2):
                    n = c * 2 + j
                    for hg in hgs:
                        q = qs[hg]
                        ps = ps_v.tile([128, 512], F32, tag="pv")
                        for kc in range(2):
                            nc.tensor.matmul(ps[:], wv_sb[:, hg, kc, :],
                                             ft[:, kc, j * 512:(j + 1) * 512],
                                             start=(kc == 0), stop=(kc == 1))
                        for dxi, sh in ((0, 0), (1, 1), (2, 64), (3, 65)):
                            if sh == 0:
                                o, i = q[:, n * 512:(n + 1) * 512, 0], ps[:]
                            elif n == 0:
                                o, i = q[:, 0:512 - sh, dxi], ps[:, sh:512]
                            else:
                                o = q[:, n * 512 - sh:(n + 1) * 512 - sh, dxi]
                                i = ps[:]
                            if dxi < 2:
                                nc.scalar.activation(out=o, in_=i,
                                                     func=Act.Copy)
                            else:
                                nc.vector.tensor_copy(out=o, in_=i)

        # ---------------- prep phases (per b) ----------------
        def prep_a(b):
            """fsh load; offset+size convs -> packed offp/szbp [128,2,LSH]."""
            s = st.setdefault(b, {})
            fsh = fshp.tile([128, 2, LSH], F32R, tag="fsh")
            dmaa(fsh[:], P['fsh'].ap()[:, b, :, :])
            s['fsh'] = fsh
            offp = prep.tile([128, 2, LSH], F32, tag="offp")
            szbp = prep.tile([128, 2, LSH], F32, tag="szbp")
            s['offp'], s['szbp'] = offp, szbp
            for xy in range(2):
                ps = ps_p.tile([128, 512], F32, tag="pp", name="psz")
                for kc in range(2):
                    nc.tensor.matmul(ps[0:8, :], wsz_sb[:, xy, kc, :],
                                     fsh[:, kc, :], start=(kc == 0),
                                     stop=(kc == 1))
                szs = gm.tile([8, LSH], F32, tag="szs")
                nc.scalar.activation(out=szs[:], in_=ps[0:8, :],
                                     func=Act.Sigmoid,
                                     bias=bsz_sb[:, xy:xy + 1], scale=1.0)
                nc.vector.tensor_scalar(out=szs[:], in0=szs[:], scalar1=0.75,
                                        scalar2=0.25, op0=Alu.min, op1=Alu.max)
                psb = ps_p.tile([128, 512], F32, tag="pp", name="psb")
                nc.tensor.matmul(psb[:], sel8_sb[:], szs[:],
                                 start=True, stop=True)
                nc.vector.tensor_copy(out=szbp[:, xy, :], in_=psb[:])
                ps2 = ps_p.tile([128, 512], F32, tag="pp", name="po")
                for kc in range(2):
                    nc.tensor.matmul(ps2[:], woff_sb[:, xy, kc, :],
                                     fsh[:, kc, :], start=(kc == 0),
                                     stop=(kc == 1))
                nc.scalar.activation(out=offp[:, xy, :], in_=ps2[:],
                                     func=Act.Sigmoid,
                                     bias=boff_sb[:, xy:xy + 1], scale=1.0)

        def prep_c(b):
            """grid -> floor -> flat y0 idx (packed x|y in one [128,2,LSH])."""
            s = st[b]
            offp, szbp = s['offp'], s['szbp']
            o2 = offp[:].rearrange("p a b -> p (a b)")
            s2 = szbp[:].rearrange("p a b -> p (a b)")
            cf = gm.tile([128, 2, LSH], F32, tag="cf")
            c2 = cf[:].rearrange("p a b -> p (a b)")
            ci = gm.tile([128, 2, LSH], I16, tag="ci")
            i2 = ci[:].rearrange("p a b -> p (a b)")
            msk = gm.tile([128, 2, LSH], F32, tag="msk")
            m2 = msk[:].rearrange("p a b -> p (a b)")
            nc.vector.tensor_scalar(out=o2, in0=o2, scalar1=-0.5,
                                    scalar2=None, op0=Alu.add)
            nc.vector.tensor_tensor(out=o2, in0=o2, in1=s2, op=Alu.mult)
            nc.vector.tensor_tensor(
                out=o2, in0=o2,
                in1=cen_sb[:].rearrange("p a b -> p (a b)"), op=Alu.add)
            nc.vector.tensor_scalar(out=o2, in0=o2, scalar1=float(W - 1),
                                    scalar2=0.0, op0=Alu.min, op1=Alu.max)
            # f32->i16 conversion rounds to nearest on HW: compare-and-
            # subtract corrects round-up cases back to floor
            nc.vector.tensor_copy(out=i2, in_=o2)
            nc.vector.tensor_copy(out=c2, in_=i2)
            nc.vector.tensor_tensor(out=m2, in0=c2, in1=o2, op=Alu.is_gt)
            nc.vector.tensor_tensor(out=c2, in0=c2, in1=m2, op=Alu.subtract)
            nc.vector.tensor_tensor(out=o2, in0=o2, in1=c2, op=Alu.subtract)
            # flat y0 = y0f*W + x0f -> reuse szbp x-half as scratch, fi -> ci
            fl = szbp[:, 0, :]
            nc.vector.tensor_scalar(out=fl, in0=cf[:, 1, :],
                                    scalar1=float(W), scalar2=None,
                                    op0=Alu.mult)
            nc.vector.tensor_tensor(out=fl, in0=fl, in1=cf[:, 0, :],
                                    op=Alu.add)
            fi = gm.tile([128, LSH], I16, tag="fi")
            nc.vector.tensor_copy(out=fi[:], in_=fl)
            s['fi'] = fi
            # wx/wy in offp halves; cf/msk slots free for prep_d reuse
            s['cf'], s['msk'] = cf, msk

        def prep_cs(b, k):
            """Wrapped y0 idx stores for l-block k + clean idxw loads +
            DVE-derived y1 idx tiles."""
            s = st[b]
            fi = s['fi']
            for hg in range(2):
                # one store for the whole head-group: partitions (hh,p) merge
                # to a single stride-16 dim in [b][lq][hh][p][lhi]
                dmas(AP(tensor=idxd[(k, hg)], offset=b * 16384,
                        ap=[[16, 64], [1, 16], [1024, 16]]),
                     fi[hg * 64:(hg + 1) * 64, k * 256:(k + 1) * 256])
                ix = idxwp.tile([128, 256], I16, tag=f"ix{hg}{k}",
                                name=f"ix{b}{hg}{k}")
                st[('ix', b, hg, k)] = ix
                for hh in range(4):
                    # 32 contiguous partitions (dup 2 x lq 16) per load; the
                    # dup replication is a stride-0 source dim
                    dmaa(ix[hh * 32:(hh + 1) * 32, :],
                         AP(tensor=idxd[(k, hg)],
                            offset=b * 16384 + hh * 256,
                            ap=[[0, 2], [1024, 16], [1, 256]]))

        def prep_b(b):
            """attn conv (pixel-major) + softmax + transpose -> aT."""
            s = st[b]
            fsh = s['fsh']
            aT = prep.tile([128, LSH], F32, tag="aT")
            s['aT'] = aT
            for lb in range(LSH // 128):
                ps = ps_p.tile([128, 128], F32, tag="pp", name="pa")
                for kc in range(2):
                    nc.tensor.matmul(ps[:], fsh[:, kc, lb * 128:(lb + 1) * 128],
                                     watt_sb[:, kc, :], start=(kc == 0),
                                     stop=False)
                nc.tensor.matmul(ps[:], ones_sb[:], batt_sb[:],
                                 start=False, stop=True)
                ae = gm.tile([128, 8, 16], F32, tag="ae")
                nc.scalar.activation(out=ae[:], in_=ps[:], func=Act.Exp)
                ssum = gm.tile([128, 8, 1], F32, tag="ssum")
                nc.vector.tensor_reduce(out=ssum[:], in_=ae[:],
                                        axis=mybir.AxisListType.X, op=Alu.add)
                nc.vector.reciprocal(out=ssum[:], in_=ssum[:])
                ssa = ssum[:]
                nc.vector.tensor_tensor(
                    out=ae[:], in0=ae[:],
                    in1=AP(tensor=ssa.tensor, offset=ssa.offset,
                           ap=[ssa.ap[0], [1, 8], [0, 16]]),
                    op=Alu.mult)
                pst = ps_p.tile([128, 128], F32, tag="pp", name="pt")
                nc.tensor.transpose(pst[:], ae[:].rearrange("p a b -> p (a b)"),
                                    id_sb[:])
                nc.scalar.activation(out=aT[:, lb * 128:(lb + 1) * 128],
                                     in_=pst[:], func=Act.Copy)

        def prep_d(b):
            """u = attn * bilinear -> upair [128, tap, l, dx] -> DRAM."""
            s = st[b]
            offp, cf, msk, aT = s['offp'], s['cf'], s['msk'], s['aT']
            wx, wy = offp[:, 0, :], offp[:, 1, :]
            omx, omy = msk[:, 0, :], msk[:, 1, :]
            ay0, ay1 = cf[:, 0, :], cf[:, 1, :]
            nc.vector.tensor_scalar(out=omx, in0=wx, scalar1=-1.0,
                                    scalar2=1.0, op0=Alu.mult, op1=Alu.add)
            nc.vector.tensor_scalar(out=omy, in0=wy, scalar1=-1.0,
                                    scalar2=1.0, op0=Alu.mult, op1=Alu.add)
            nc.vector.tensor_tensor(out=ay0, in0=aT[:], in1=omy, op=Alu.mult)
            nc.vector.tensor_tensor(out=ay1, in0=aT[:], in1=wy, op=Alu.mult)
            uq = gm.tile([128, LSH, 4], BF16, tag="uq")
            for dxi, (yf, xf) in enumerate(((ay0, omx), (ay0, wx),
                                            (ay1, omx), (ay1, wx))):
                nc.vector.tensor_tensor(out=uq[:, :, dxi], in0=yf,
                                        in1=xf, op=Alu.mult)
            dmaa(AP(tensor=ud, offset=b * 128 * 2048,
                    ap=[[2048, 128], [1, 2048]]),
                 uq[:].rearrange("p a b -> p (a b)"))

        # ---------------- gather + combine ----------------
        pso = {}
        cnt = {}
        ubc_rr = [0]

        def gblock(b, hg, k):
            quad = mem_t[(b, hg)]
            idxw = st[('ix', b, hg, k)]
            for oc in range(2):
                if (b, k, oc) not in pso:
                    pso[(b, k, oc)] = ps_o.tile([128, 256], F32,
                                                tag=f"po{k}{oc}",
                                                name=f"po{b}{k}{oc}")
                    cnt[(b, k, oc)] = 0
            if dve_combine:
                # p-half d4 gathers -> permuted multiply into gu (l,p,dx) ->
                # in-place bf16 add tree over (p,dx) -> 2 out_proj matmuls
                gu = gup.tile([128, 256, 16, 4], BF16, tag="gu", bufs=1)
                gua = gu[:]
                if gather8:
                    # one 4096-idx gather per block: per-call ucode fixed
                    # cost dominates, so fewer larger calls win even with
                    # bufs=1 (next gather waits this block's 4 mults)
                    g8 = gathp.tile([128, 4096, 4], BF16, tag="g8", bufs=1)
                    if stub_gather:
                        nc.gpsimd.ap_gather(
                            g8[:, 0:16, :],
                            quad[:].rearrange("p a b -> p (a b)"),
                            idxw[:, 0:1], channels=128,
                            num_elems=L, d=4, num_idxs=16)
                    else:
                        nc.gpsimd.ap_gather(
                            g8[:], quad[:].rearrange("p a b -> p (a b)"),
                            idxw[:], channels=128,
                            num_elems=L, d=4, num_idxs=4096)
                    ghs = [g8, g8]
                else:
                    ghs = []
                    for jh in range(2):
                        gh = gathp.tile([128, 2048, 4], BF16, tag="g4h")
                        ghs.append(gh)
                        if stub_gather:
                            nc.gpsimd.ap_gather(
                                gh[:, 0:16, :],
                                quad[:].rearrange("p a b -> p (a b)"),
                                idxw[:, 0:1], channels=128,
                                num_elems=L, d=4, num_idxs=16)
                        else:
                            nc.gpsimd.ap_gather(
                                gh[:], quad[:].rearrange("p a b -> p (a b)"),
                                idxw[:, jh * 128:(jh + 1) * 128],
                                channels=128, num_elems=L, d=4,
                                num_idxs=2048)
                for jq in range(4):
                    ubc = ubcp.tile([128, 4096], BF16, tag="ubc")
                    for hh in range(4):
                        r = ubc_rr[0] % 2
                        ubc_rr[0] += 1
                        dmaq = (dmas, dmaa)[r]
                        if stub_ubc and hh > 0:
                            continue
                        dmaq(ubc[hh * 32:(hh + 1) * 32, :],
                             AP(tensor=ud,
                                offset=(b * 128 + (hg * 4 + hh) * 16
                                        + jq * 4) * 2048 + k * 1024,
                                ap=[[0, 32], [2048, 4], [1, 1024]]))
                    gh = ghs[jq // 2]
                    jo = jq if gather8 else jq % 2
                    sl = gh[:, jo * 1024:(jo + 1) * 1024, :]
                    nc.vector.tensor_tensor(
                        out=AP(tensor=gua.tensor,
                               offset=gua.offset + jq * 16,
                               ap=[gua.ap[0], [4, 4], [64, 256], [1, 4]]),
                        in0=sl.rearrange("p a b -> p (a b)"),
                        in1=ubc[:], op=Alu.mult)

                def tadd(o, i1):
                    nc.vector.tensor_tensor(out=o, in0=o, in1=i1, op=Alu.add)
                tadd(gu[:, :, :, 0:2], gu[:, :, :, 2:4])
                tadd(gu[:, :, 0:8, 0:2], gu[:, :, 8:16, 0:2])
                tadd(gu[:, :, 0:4, 0:2], gu[:, :, 4:8, 0:2])
                tadd(gu[:, :, 0:2, 0:2], gu[:, :, 2:4, 0:2])
                tadd(gu[:, :, 0:1, 0:2], gu[:, :, 1:2, 0:2])
                tadd(gu[:, :, 0:1, 0:1], gu[:, :, 0:1, 1:2])
                rhs = AP(tensor=gua.tensor, offset=gua.offset,
                         ap=[gua.ap[0], [64, 256]])
                for oc in range(2):
                    c = cnt[(b, k, oc)]
                    nc.tensor.matmul(pso[(b, k, oc)][:],
                                     wout_sb[:, hg, oc, :], rhs,
                                     start=(c == 0), stop=(c == 1))
                    cnt[(b, k, oc)] = c + 1
                return
            g = gathp.tile([128, 4096, 4], BF16, tag="g4")
            if gather_f32:
                # f32-pair view: one gathered "element" = 2 packed bf16 taps,
                # so d=2 f32 halves the gather's free-size cost.
                qf = quad[:].rearrange("p a b -> p (a b)").bitcast(F32)
                if stub_gather:
                    nc.gpsimd.ap_gather(
                        g[:, 0:16, :].rearrange(
                            "p a b -> p (a b)").bitcast(F32),
                        qf, idxw[:, 0:1], channels=128,
                        num_elems=L, d=2, num_idxs=16)
                else:
                    nc.gpsimd.ap_gather(
                        g[:].rearrange("p a b -> p (a b)").bitcast(F32),
                        qf, idxw[:], channels=128,
                        num_elems=L, d=2, num_idxs=4096)
            elif stub_gather:
                nc.gpsimd.ap_gather(
                    g[:, 0:16, :], quad[:].rearrange("p a b -> p (a b)"),
                    idxw[:, 0:1], channels=128,
                    num_elems=L, d=4, num_idxs=16)
            else:
                nc.gpsimd.ap_gather(
                    g[:], quad[:].rearrange("p a b -> p (a b)"),
                    idxw[:], channels=128,
                    num_elems=L, d=4, num_idxs=4096)
            nj, pq = (2, 8) if ubc_half else (4, 4)
            for jq in range(nj):      # u-broadcasts: p in [pq*jq, pq*(jq+1))
                ubc = ubcp.tile([128, 1024 * pq], BF16, tag="ubc",
                                bufs=1 if ubc_half else 2)
                for hh in range(4):
                    # balance broadcast traffic across SP/ACT/Pool DMA queues
                    r = ubc_rr[0] % 2
                    ubc_rr[0] += 1
                    dmaq = (dmas, dmaa)[r]
                    if stub_ubc and hh > 0:
                        continue
                    dmaq(ubc[hh * 32:(hh + 1) * 32, :],
                         AP(tensor=ud,
                            offset=(b * 128 + (hg * 4 + hh) * 16 + jq * pq)
                            * 2048 + k * 1024,
                            ap=[[0, 32], [2048, pq], [1, 1024]]))
                sl = g[:, jq * pq * 256:(jq + 1) * pq * 256, :].rearrange(
                    "p a b -> p (a b)")
                nc.vector.tensor_tensor(out=sl, in0=sl, in1=ubc[:],
                                        op=Alu.mult)
            gap = g[:]
            for oc in range(2):
                for p in range(16):
                    for dxi in range(4):
                        rhs = AP(tensor=gap.tensor,
                                 offset=gap.offset + p * 1024 + dxi,
                                 ap=[gap.ap[0], [4, 256]])
                        c = cnt[(b, k, oc)]
                        nc.tensor.matmul(
                            pso[(b, k, oc)][:],
                            wout_sb[:, hg, oc, :], rhs,
                            start=(c == 0), stop=(c == 127))
                        cnt[(b, k, oc)] = c + 1

        def finalize(b, k):
            for oc in range(2):
                o_sb = outp.tile([128, 256], F32, tag="osb")
                nc.scalar.activation(out=o_sb[:], in_=pso[(b, k, oc)][:],
                                     func=Act.Identity,
                                     bias=obias_sb[:, oc:oc + 1], scale=1.0)
                dmaa(AP(tensor=out_e,
                        offset=((b * 2 + oc) * 128) * LSH + k * 256,
                        ap=[[LSH, 128], [1, 256]]), o_sb[:])

        # ---------------- emission schedule ----------------
        # idx path + hg0 quad first so the Pool engine's first gather can
        # start as early as possible; everything else fills in under the
        # gather stream.
        prep_a(0)
        prep_c(0)
        prep_cs(0, 0)
        conv_sec(0, (0,))
        prep_b(0)
        prep_d(0)
        prep_cs(0, 1)
        gblock(0, 0, 0)
        conv_sec(0, (1,))
        gblock(0, 0, 1)
        prep_a(1)
        prep_b(1)
        prep_c(1)
        prep_d(1)
        conv_sec(1, (0,))
        gblock(0, 1, 0)
        finalize(0, 0)
        prep_cs(1, 0)
        gblock(0, 1, 1)
        finalize(0, 1)
        prep_cs(1, 1)
        conv_sec(1, (1,))
        gblock(1, 0, 0)
        gblock(1, 0, 1)
        gblock(1, 1, 0)
        finalize(1, 0)
        gblock(1, 1, 1)
        finalize(1, 1)

    nc.compile()
    _GRAPH_CACHE[key] = nc
    return nc


def stage_inputs(inputs, core):
    """Build the per-core in_map (all arrays pre-laid-out for plain DMAs)."""
    bf16 = ml_dtypes.bfloat16
    feat = np.ascontiguousarray(
        np.asarray(inputs['feat_sd'], np.float32).reshape(B, C, L))
    lo = core * LSH
    WvT = np.asarray(inputs['value_proj_w'], np.float32).T.copy()
    WoffT = np.asarray(inputs['anchor_deform_w'], np.float32).T.copy()
    WattT = np.asarray(inputs['anchor_att_w'], np.float32).T.copy()
    WszT = np.asarray(inputs['size_deform_w'], np.float32).T.copy()
    WoutT = np.asarray(inputs['out_proj_w'], np.float32).T.copy()
    boff = np.asarray(inputs['anchor_deform_b'], np.float32)
    bsz = np.asarray(inputs['size_deform_b'], np.float32)
    bv = np.asarray(inputs['value_proj_b'], np.float32)
    bn_s = (np.asarray(inputs['bn_gamma'], np.float32)
            / np.sqrt(np.float32(1.0 + 1e-5)))
    beta = np.asarray(inputs['bn_beta'], np.float32)
    WoutT_sc = WoutT * bn_s[None, :]
    obias = (bv @ WoutT_sc + beta).reshape(2, 128).T
    sel8 = np.zeros((8, 128), np.float32)
    for h in range(8):
        sel8[h, h * 16:(h + 1) * 16] = float(W - 1)
    cols = (np.arange(W) + 0.5) / (W + EPS)
    rows = (np.arange(H) + 0.5) / (H + EPS)
    cx = np.tile(cols, H)[lo:lo + LSH].astype(np.float32)
    cy = np.repeat(rows, W)[lo:lo + LSH].astype(np.float32)
    cen2 = np.stack([np.broadcast_to(cx, (128, LSH)),
                     np.broadcast_to(cy, (128, LSH))], axis=1) * (W - 1.0)
    # woff/wsz packed: xy-interleaved output channels split into x|y planes
    woff = np.stack([WoffT[:, 0::2], WoffT[:, 1::2]],
                    axis=1)                      # [256, 2, 128]
    woff_t = woff.reshape(2, 128, 2, 128).transpose(1, 2, 0, 3)
    wsz = np.stack([WszT[:, 0::2], WszT[:, 1::2]], axis=1)  # [256, 2, 8]
    wsz_t = wsz.reshape(2, 128, 2, 8).transpose(1, 2, 0, 3)
    fr = feat.reshape(B, 2, 128, L)
    m = {
        'featb': np.ascontiguousarray(
            fr.transpose(2, 0, 1, 3)).astype(bf16),
        'fsh': np.ascontiguousarray(
            fr[:, :, :, lo:lo + LSH].transpose(2, 0, 1, 3)),
        'wv_t': np.ascontiguousarray(
            WvT.reshape(2, 128, 2, 128).transpose(1, 2, 0, 3)).astype(bf16),
        'woff_t': np.ascontiguousarray(woff_t),
        'boff_p': np.ascontiguousarray(
            np.stack([boff[0::2], boff[1::2]], axis=1)),
        'wsz_t': np.ascontiguousarray(wsz_t),
        'bsz_p': np.ascontiguousarray(
            np.stack([bsz[0::2], bsz[1::2]], axis=1)),
        'sel8': sel8,
        'watt_t': np.ascontiguousarray(
            WattT.reshape(2, 128, 128).transpose(1, 0, 2)),
        'batt_r': np.asarray(inputs['anchor_att_b'],
                             np.float32).reshape(1, 128),
        'ones1': np.ones((1, 128), np.float32),
        'ident': np.eye(128, dtype=np.float32),
        'wout_t': np.ascontiguousarray(
            WoutT_sc.reshape(2, 128, 2, 128).transpose(1, 0, 2, 3)
        ).astype(bf16),
        'obias': np.ascontiguousarray(obias),
        'cen2': np.ascontiguousarray(cen2),
    }
    return m


def kernel(**inputs):
    nc = build_graph()
    in_maps = [stage_inputs(inputs, i) for i in range(NCORES)]
    res = run_bass_kernel_spmd(nc, in_maps, core_ids=list(range(NCORES)))
    shards = [res.results[i]['out'].reshape(B, C, LSH) for i in range(NCORES)]
    full = np.concatenate(shards, axis=2).reshape(B, C, H, W)
    return full.astype(np.float32)



# revision 47
# speedup vs baseline: 2.1966x; 1.0188x over previous
"""Trainium2 Bass kernel for nn_AnchorDeformAtt (deformable anchor attention).

Sharding: spatial L-shard across 8 cores -- core i handles pixels
l in [512i, 512(i+1)) for BOTH batches and ALL heads. Zero collectives;
the host concatenates per-core output shards.

Design (the gather is ~75% of runtime; everything else hides under it):
  - Memory: quad rows (m[j], m[j+1], m[j+64], m[j+65]) bf16, so ONE
    index per sample point fetches all 4 bilinear taps (d=4 bf16 --
    measured faster on HW than the equivalent d=2 f32-pair view).
  - The GPSIMD engine runs ONLY ap_gathers: Pool-issued DMAs go through
    SWDGE holding the engine and head-of-line-block the gather queue
    (measured ~370us); all DMA issue lives on the SP/ACT queues.
  - Value conv in bf16 (feat staged bf16 host-side); bias folded out
    (softmax x bilinear weights sum to 1 => Wout @ bv is a constant
    output bias, merged with bn_beta host-side; BN scale folded into
    Wout). Prep convs in float32r (tf32 rate).
  - Gather stream (pwrap): the 16-partition wrap dim is the sample
    POINT p (j%16 = p), not l%16. The idx rows are then exactly fi's
    (h,p) partition rows: 8 SBUF->SBUF row-replication DMAs per (b,hg)
    replace the whole DRAM transpose round-trip (whose 2B-scattered
    stores cost 7.2us each and stalled the fill through DMA-counter
    aliasing). Bonus: gather output lands directly in (l,p,dx)-major
    order, so the u-multiply runs IN-PLACE on the gathered tile (no
    permuted write, no separate gu tile) -- the freed 32KB enables one
    4096-idx gather per block with double buffering (8 calls instead
    of 16; measured -360us total vs the DRAM-wrap 16-call version).
  - f32->i16 index conversion ROUNDS on HW (CoreSim truncates): the
    is_gt/subtract pass after the copy restores floor semantics.
  - u = attn * bilinear weights staged as [b][(h,p)][l][dx4],
    broadcast to each head's 32 channels with 3-dim-AP DMAs (cheap:
    fully hidden under the gather once off the Pool queue).
  - Combine (dve_combine): the gathered [128ch, l, p, dx] tile is
    multiplied in-place by u per p-quarter (strided 4-dim in1 AP reads
    the p-major ubc tile in (l,p,dx) order), then an in-place bf16
    pairwise add tree reduces (p,dx) -- 7 DVE ops replace 128
    accumulating matmuls; out_proj is 2 matmuls per gblock (stride-64
    rhs on the reduced column). ~200us faster on HW than PE-combine.
  - Emission schedule: idx path + hg0 quad first so the first gather
    starts as early as possible; conv_sec(1,(0,)) is emitted before
    gblock(0,1,0) so b1's quad copies queue on DVE ahead of the
    gather-dependent mults (removes a ~40us stall at the b transition).
    conv_sec(1,(1,)) must stay after gblock(0,1,1): its memset waits on
    quad01's buffer, whose release needs those mults (DVE deadlock
    otherwise).
  - Measured dead ends (same-process A/B on HW): d=2 f32-pair gather
    (+100us despite half the elements), coarser ubc tiles with bufs=1
    (+200us), one 4096-idx gather per block with gathp bufs=1 (+220us:
    the mult-drain serialization outweighs the ~13us/call fixed cost),
    ubc issue pinned to one queue, and consts alternated onto ACT
    (both regress the fill in the timeline model).
"""
from contextlib import ExitStack

import numpy as np
import ml_dtypes

import concourse.bass as bass
import concourse.mybir as mybir
import concourse.tile as tile
from concourse import bacc
from concourse.bass_utils import run_bass_kernel_spmd

NH, NP = 8, 16
B, C, H, W = 2, 256, 64, 64
L = H * W            # 4096
NCORES = 8
LSH = L // NCORES    # 512
LPAD = L + 64        # pairs-memory rows (y1 tap reads idx+64)
EPS = 1e-6
F32 = mybir.dt.float32
BF16 = mybir.dt.bfloat16
I16 = mybir.dt.int16
F32R = mybir.dt.float32r
FP8 = mybir.dt.float8e4

_GRAPH_CACHE = {}

PARAM_SPECS = {
    'featb': ([128, B, 2, L], BF16),
    'fsh': ([128, B, 2, LSH], F32R),
    'wv_t': ([128, 2, 2, 128], BF16),    # K, hg, kc, M
    'woff_t': ([128, 2, 2, 128], F32R),   # K, xy, kc, M
    'boff_p': ([128, 2], F32),           # per-partition bias, xy
    'wsz_t': ([128, 2, 2, 8], F32R),      # K, xy, kc, 8
    'bsz_p': ([8, 2], F32),
    'sel8': ([8, 128], F32),             # sel8[h, h*16+p] = 1
    'watt_t': ([128, 2, 128], F32R),      # K, kc, N
    'batt_r': ([1, 128], F32),
    'ones1': ([1, 128], F32),
    'ident': ([128, 128], F32),
    'wout_t': ([128, 2, 2, 128], BF16),  # K, hg, oc, M (bn-scaled)
    'obias': ([128, 2], F32),            # (Wout_sc @ bv + beta) as [m, oc]
    'cen2': ([128, 2, LSH], F32),        # packed (x|y) centers
}


def build_graph(stub_gather=False, gather_f32=False, ubc_half=False,
                dve_combine=True, stub_ubc=False, gather8=False,
                qsplit=False, pwrap=True):
    key = (stub_gather, gather_f32, ubc_half, dve_combine, stub_ubc, gather8,
           qsplit, pwrap)
    if key in _GRAPH_CACHE:
        return _GRAPH_CACHE[key]

    nc = bacc.Bacc("TRN2", target_bir_lowering=False, debug=False,
                   num_devices=NCORES)
    dp = nc.declare_dram_parameter
    P = {n: dp(n, s, dt, isOutput=False) for n, (s, dt) in PARAM_SPECS.items()}
    out_e = dp("out", [B, 2, 128, LSH], F32, isOutput=True)

    # wrapped y0 idx staging, one tensor per (k, hg): [b][lq][hh][p][lhi]
    # (lq-major so the (hh,p) partition dims merge to one stride-16 dim)
    idxd = {(k, hg): nc.dram_tensor(f"idxd{k}{hg}", [B, 16, 4, 16, 16], I16)
            for k in range(2) for hg in range(2)}
    # u staging: [b][(h,p) 128][tap 2][l 512][dx 2]
    ud = nc.dram_tensor("ud", [B, 128, 2, LSH, 2], BF16)

    AP = bass.AP
    Act = mybir.ActivationFunctionType
    Alu = mybir.AluOpType

    with tile.TileContext(nc) as tc, ExitStack() as ctx:
        consts = ctx.enter_context(tc.tile_pool(name="consts", bufs=1))
        featp = ctx.enter_context(tc.tile_pool(name="featp", bufs=2))
        fshp = ctx.enter_context(tc.tile_pool(name="fshp", bufs=1))
        memp = ctx.enter_context(tc.tile_pool(name="memp", bufs=1))
        prep = ctx.enter_context(tc.tile_pool(name="prep", bufs=1))
        gm = ctx.enter_context(tc.tile_pool(name="gm", bufs=1))
        idxwp = ctx.enter_context(tc.tile_pool(name="idxwp", bufs=2))
        ubcp = ctx.enter_context(tc.tile_pool(name="ubcp", bufs=2))
        gathp = ctx.enter_context(tc.tile_pool(name="gathp", bufs=2))
        gup = ctx.enter_context(tc.tile_pool(name="gup", bufs=1))
        outp = ctx.enter_context(tc.tile_pool(name="outp", bufs=2))
        ps_v = ctx.enter_context(tc.tile_pool(name="ps_v", bufs=2, space="PSUM"))
        ps_p = ctx.enter_context(tc.tile_pool(name="ps_p", bufs=2, space="PSUM"))
        ps_o = ctx.enter_context(tc.tile_pool(name="ps_o", bufs=1, space="PSUM"))

        def dmas(out, in_):          # SP queue: consts, ft, idx stores, ubc
            nc.sync.dma_start(out=out, in_=in_)

        def dmaa(out, in_):          # ACT queue: fsh, idxw loads, uq, out
            nc.scalar.dma_start(out=out, in_=in_)

        # ---------------- constants ----------------
        crr = [0]

        def cload(name, q=None):
            shape, dt = PARAM_SPECS[name]
            t = consts.tile(list(shape), dt, tag=name, name=f"c_{name}")
            if q is None:
                q = (dmas, dmaa)[crr[0] % 2] if qsplit else dmas
                crr[0] += 1
            q(t[:], P[name].ap())
            return t

        def dmag(out, in_):
            nc.gpsimd.dma_start(out=out, in_=in_)

        wv_sb = cload('wv_t')
        woff_sb = cload('woff_t')
        boff_sb = cload('boff_p')
        wsz_sb = cload('wsz_t')
        bsz_sb = cload('bsz_p')
        sel8_sb = cload('sel8')
        watt_sb = cload('watt_t')
        batt_sb = cload('batt_r')
        ones_sb = cload('ones1')
        id_sb = cload('ident')
        wout_sb = cload('wout_t')
        obias_sb = cload('obias')
        cen_sb = cload('cen2')

        mem_t = {}
        st = {}

        # ---------------- value conv (bf16 quads, no bias) ----------------
        def conv_sec(b, hgs=(0, 1), act_all=False):
            qs = {}
            for hg in hgs:
                q = memp.tile([128, L, 4], BF16, tag="quad",
                              name=f"quad{b}{hg}", bufs=2)
                mem_t[(b, hg)] = q
                nc.vector.memset(q[:, L - 65:, :], 0.0)
                qs[hg] = q
            for c in range(4):
                ft = featp.tile([128, 2, 1024], BF16, tag="ft")
                ftq = (dmas, dmaa)[c % 2]
                ftq(ft[:], AP(tensor=P['featb'], offset=b * 2 * L + c * 1024,
                              ap=[[B * 2 * L, 128], [L, 2], [1, 1024]]))
                for j in range(2):
                    n = c * 2 + j
                    for hg in hgs:
                        q = qs[hg]
                        ps = ps_v.tile([128, 512], F32, tag="pv")
                        for kc in range(2):
                            nc.tensor.matmul(ps[:], wv_sb[:, hg, kc, :],
                                             ft[:, kc, j * 512:(j + 1) * 512],
                                             start=(kc == 0), stop=(kc == 1))
                        for dxi, sh in ((0, 0), (1, 1), (2, 64), (3, 65)):
                            if sh == 0:
                                o, i = q[:, n * 512:(n + 1) * 512, 0], ps[:]
                            elif n == 0:
                                o, i = q[:, 0:512 - sh, dxi], ps[:, sh:512]
                            else:
                                o = q[:, n * 512 - sh:(n + 1) * 512 - sh, dxi]
                                i = ps[:]
                            if dxi < 2 or act_all:
                                nc.scalar.activation(out=o, in_=i,
                                                     func=Act.Copy)
                            else:
                                nc.vector.tensor_copy(out=o, in_=i)

        # ---------------- prep phases (per b) ----------------
        def prep_fsh(b):
            s = st.setdefault(b, {})
            fsh = fshp.tile([128, 2, LSH], F32R, tag="fsh")
            dmaa(fsh[:], P['fsh'].ap()[:, b, :, :])
            s['fsh'] = fsh

        def prep_a(b):
            """offset+size convs -> packed offp/szbp [128,2,LSH]."""
            s = st[b]
            fsh = s['fsh']
            offp = prep.tile([128, 2, LSH], F32, tag="offp")
            szbp = prep.tile([128, 2, LSH], F32, tag="szbp")
            s['offp'], s['szbp'] = offp, szbp
            for xy in range(2):
                ps = ps_p.tile([128, 512], F32, tag="pp", name="psz")
                for kc in range(2):
                    nc.tensor.matmul(ps[0:8, :], wsz_sb[:, xy, kc, :],
                                     fsh[:, kc, :], start=(kc == 0),
                                     stop=(kc == 1))
                szs = gm.tile([8, LSH], F32, tag="szs")
                nc.scalar.activation(out=szs[:], in_=ps[0:8, :],
                                     func=Act.Sigmoid,
                                     bias=bsz_sb[:, xy:xy + 1], scale=1.0)
                nc.vector.tensor_scalar(out=szs[:], in0=szs[:], scalar1=0.75,
                                        scalar2=0.25, op0=Alu.min, op1=Alu.max)
                psb = ps_p.tile([128, 512], F32, tag="pp", name="psb")
                nc.tensor.matmul(psb[:], sel8_sb[:], szs[:],
                                 start=True, stop=True)
                nc.vector.tensor_copy(out=szbp[:, xy, :], in_=psb[:])
                ps2 = ps_p.tile([128, 512], F32, tag="pp", name="po")
                for kc in range(2):
                    nc.tensor.matmul(ps2[:], woff_sb[:, xy, kc, :],
                                     fsh[:, kc, :], start=(kc == 0),
                                     stop=(kc == 1))
                nc.scalar.activation(out=offp[:, xy, :], in_=ps2[:],
                                     func=Act.Sigmoid,
                                     bias=boff_sb[:, xy:xy + 1], scale=1.0)

        def prep_c(b):
            """grid -> floor -> flat y0 idx (packed x|y in one [128,2,LSH])."""
            s = st[b]
            offp, szbp = s['offp'], s['szbp']
            o2 = offp[:].rearrange("p a b -> p (a b)")
            s2 = szbp[:].rearrange("p a b -> p (a b)")
            cf = gm.tile([128, 2, LSH], F32, tag="cf")
            c2 = cf[:].rearrange("p a b -> p (a b)")
            ci = gm.tile([128, 2, LSH], I16, tag="ci")
            i2 = ci[:].rearrange("p a b -> p (a b)")
            msk = gm.tile([128, 2, LSH], F32, tag="msk")
            m2 = msk[:].rearrange("p a b -> p (a b)")
            nc.vector.tensor_scalar(out=o2, in0=o2, scalar1=-0.5,
                                    scalar2=None, op0=Alu.add)
            nc.vector.tensor_tensor(out=o2, in0=o2, in1=s2, op=Alu.mult)
            nc.vector.tensor_tensor(
                out=o2, in0=o2,
                in1=cen_sb[:].rearrange("p a b -> p (a b)"), op=Alu.add)
            nc.vector.tensor_scalar(out=o2, in0=o2, scalar1=float(W - 1),
                                    scalar2=0.0, op0=Alu.min, op1=Alu.max)
            # f32->i16 conversion rounds to nearest on HW: compare-and-
            # subtract corrects round-up cases back to floor
            nc.vector.tensor_copy(out=i2, in_=o2)
            nc.vector.tensor_copy(out=c2, in_=i2)
            nc.vector.tensor_tensor(out=m2, in0=c2, in1=o2, op=Alu.is_gt)
            nc.vector.tensor_tensor(out=c2, in0=c2, in1=m2, op=Alu.subtract)
            nc.vector.tensor_tensor(out=o2, in0=o2, in1=c2, op=Alu.subtract)
            # flat y0 = y0f*W + x0f -> reuse szbp x-half as scratch, fi -> ci
            fl = szbp[:, 0, :]
            nc.vector.tensor_scalar(out=fl, in0=cf[:, 1, :],
                                    scalar1=float(W), scalar2=None,
                                    op0=Alu.mult)
            nc.vector.tensor_tensor(out=fl, in0=fl, in1=cf[:, 0, :],
                                    op=Alu.add)
            fi = gm.tile([128, LSH], I16, tag="fi")
            nc.vector.tensor_copy(out=fi[:], in_=fl)
            s['fi'] = fi
            # wx/wy in offp halves; cf/msk slots free for prep_d reuse
            s['cf'], s['msk'] = cf, msk

        def prep_cs(b, k, hgs=(0, 1)):
            """Wrapped y0 idx stores for l-block k + clean idxw loads +
            DVE-derived y1 idx tiles."""
            s = st[b]
            fi = s['fi']
            if pwrap:
                # p-wrapped idx streams (j%16 = point p): the idx rows ARE
                # fi's (h,p) partition rows -- pure SBUF->SBUF row
                # replication, no DRAM round-trip. One tile per (b,hg)
                # covers both k-blocks.
                if k != 0:
                    return
                for hg in hgs:
                    ix = idxwp.tile([128, 512], I16, tag=f"ixp{hg}",
                                    name=f"ixp{b}{hg}")
                    st[('ix', b, hg)] = ix
                    for hh in range(4):
                        for dup in range(2):
                            r = hh * 32 + dup * 16
                            dmaa(ix[r:r + 16, :],
                                 fi[hg * 64 + hh * 16:hg * 64 + hh * 16 + 16,
                                    :])
                return
            for hg in hgs:
                # one store for the whole head-group: partitions (hh,p) merge
                # to a single stride-16 dim in [b][lq][hh][p][lhi]
                dmas(AP(tensor=idxd[(k, hg)], offset=b * 16384,
                        ap=[[16, 64], [1, 16], [1024, 16]]),
                     fi[hg * 64:(hg + 1) * 64, k * 256:(k + 1) * 256])
                ix = idxwp.tile([128, 256], I16, tag=f"ix{hg}{k}",
                                name=f"ix{b}{hg}{k}")
                st[('ix', b, hg, k)] = ix
                for hh in range(4):
                    # 32 contiguous partitions (dup 2 x lq 16) per load; the
                    # dup replication is a stride-0 source dim
                    dmaa(ix[hh * 32:(hh + 1) * 32, :],
                         AP(tensor=idxd[(k, hg)],
                            offset=b * 16384 + hh * 256,
                            ap=[[0, 2], [1024, 16], [1, 256]]))

        def prep_b(b):
            """attn conv (pixel-major) + softmax + transpose -> aT."""
            s = st[b]
            fsh = s['fsh']
            aT = prep.tile([128, LSH], F32, tag="aT")
            s['aT'] = aT
            for lb in range(LSH // 128):
                ps = ps_p.tile([128, 128], F32, tag="pp", name="pa")
                for kc in range(2):
                    nc.tensor.matmul(ps[:], fsh[:, kc, lb * 128:(lb + 1) * 128],
                                     watt_sb[:, kc, :], start=(kc == 0),
                                     stop=False)
                nc.tensor.matmul(ps[:], ones_sb[:], batt_sb[:],
                                 start=False, stop=True)
                ae = gm.tile([128, 8, 16], F32, tag="ae")
                nc.scalar.activation(out=ae[:], in_=ps[:], func=Act.Exp)
                ssum = gm.tile([128, 8, 1], F32, tag="ssum")
                nc.vector.tensor_reduce(out=ssum[:], in_=ae[:],
                                        axis=mybir.AxisListType.X, op=Alu.add)
                nc.vector.reciprocal(out=ssum[:], in_=ssum[:])
                ssa = ssum[:]
                nc.vector.tensor_tensor(
                    out=ae[:], in0=ae[:],
                    in1=AP(tensor=ssa.tensor, offset=ssa.offset,
                           ap=[ssa.ap[0], [1, 8], [0, 16]]),
                    op=Alu.mult)
                pst = ps_p.tile([128, 128], F32, tag="pp", name="pt")
                nc.tensor.transpose(pst[:], ae[:].rearrange("p a b -> p (a b)"),
                                    id_sb[:])
                nc.scalar.activation(out=aT[:, lb * 128:(lb + 1) * 128],
                                     in_=pst[:], func=Act.Copy)

        def prep_d(b):
            """u = attn * bilinear -> upair [128, tap, l, dx] -> DRAM."""
            s = st[b]
            offp, cf, msk, aT = s['offp'], s['cf'], s['msk'], s['aT']
            wx, wy = offp[:, 0, :], offp[:, 1, :]
            omx, omy = msk[:, 0, :], msk[:, 1, :]
            ay0, ay1 = cf[:, 0, :], cf[:, 1, :]
            nc.vector.tensor_scalar(out=omx, in0=wx, scalar1=-1.0,
                                    scalar2=1.0, op0=Alu.mult, op1=Alu.add)
            nc.vector.tensor_scalar(out=omy, in0=wy, scalar1=-1.0,
                                    scalar2=1.0, op0=Alu.mult, op1=Alu.add)
            nc.vector.tensor_tensor(out=ay0, in0=aT[:], in1=omy, op=Alu.mult)
            nc.vector.tensor_tensor(out=ay1, in0=aT[:], in1=wy, op=Alu.mult)
            uq = gm.tile([128, LSH, 4], BF16, tag="uq")
            for dxi, (yf, xf) in enumerate(((ay0, omx), (ay0, wx),
                                            (ay1, omx), (ay1, wx))):
                nc.vector.tensor_tensor(out=uq[:, :, dxi], in0=yf,
                                        in1=xf, op=Alu.mult)
            dmaa(AP(tensor=ud, offset=b * 128 * 2048,
                    ap=[[2048, 128], [1, 2048]]),
                 uq[:].rearrange("p a b -> p (a b)"))

        # ---------------- gather + combine ----------------
        pso = {}
        cnt = {}
        ubc_rr = [0]

        def gblock(b, hg, k):
            quad = mem_t[(b, hg)]
            idxw = st[('ix', b, hg)] if pwrap else st[('ix', b, hg, k)]
            for oc in range(2):
                if (b, k, oc) not in pso:
                    pso[(b, k, oc)] = ps_o.tile([128, 256], F32,
                                                tag=f"po{k}{oc}",
                                                name=f"po{b}{k}{oc}")
                    cnt[(b, k, oc)] = 0
            if dve_combine and pwrap:
                # p-wrapped stream: gather output is ALREADY (l,p,dx)-major,
                # so one 4096-idx gather per block lands in-place; multiply
                # in-place per p-quarter, bf16 add tree, 2 out_proj matmuls
                gw = gathp.tile([128, 256, 16, 4], BF16, tag="gw")
                gwa = gw[:]
                idxw2 = st[('ix', b, hg)]
                if stub_gather:
                    nc.gpsimd.ap_gather(
                        AP(tensor=gwa.tensor, offset=gwa.offset,
                           ap=[gwa.ap[0], [1, 64]]),
                        quad[:].rearrange("p a b -> p (a b)"),
                        idxw2[:, 0:1], channels=128,
                        num_elems=L, d=4, num_idxs=16)
                else:
                    nc.gpsimd.ap_gather(
                        gw[:].rearrange("p a b c -> p (a b c)"),
                        quad[:].rearrange("p a b -> p (a b)"),
                        idxw2[:, k * 256:(k + 1) * 256],
                        channels=128, num_elems=L, d=4, num_idxs=4096)
                for jq in range(4):
                    ubc = ubcp.tile([128, 4096], BF16, tag="ubc")
                    for hh in range(4):
                        r = ubc_rr[0] % 2
                        ubc_rr[0] += 1
                        dmaq = (dmas, dmaa)[r]
                        if stub_ubc and hh > 0:
                            continue
                        dmaq(ubc[hh * 32:(hh + 1) * 32, :],
                             AP(tensor=ud,
                                offset=(b * 128 + (hg * 4 + hh) * 16
                                        + jq * 4) * 2048 + k * 1024,
                                ap=[[0, 32], [2048, 4], [1, 1024]]))
                    sl = gw[:, :, jq * 4:(jq + 1) * 4, :]
                    ubca = ubc[:]
                    nc.vector.tensor_tensor(
                        out=sl, in0=sl,
                        in1=AP(tensor=ubca.tensor, offset=ubca.offset,
                               ap=[ubca.ap[0], [4, 256], [1024, 4], [1, 4]]),
                        op=Alu.mult)

                def tadd2(o, i1):
                    nc.vector.tensor_tensor(out=o, in0=o, in1=i1, op=Alu.add)
                tadd2(gw[:, :, :, 0:2], gw[:, :, :, 2:4])
                tadd2(gw[:, :, 0:8, 0:2], gw[:, :, 8:16, 0:2])
                tadd2(gw[:, :, 0:4, 0:2], gw[:, :, 4:8, 0:2])
                tadd2(gw[:, :, 0:2, 0:2], gw[:, :, 2:4, 0:2])
                tadd2(gw[:, :, 0:1, 0:2], gw[:, :, 1:2, 0:2])
                tadd2(gw[:, :, 0:1, 0:1], gw[:, :, 0:1, 1:2])
                rhs = AP(tensor=gwa.tensor, offset=gwa.offset,
                         ap=[gwa.ap[0], [64, 256]])
                for oc in range(2):
                    c = cnt[(b, k, oc)]
                    nc.tensor.matmul(pso[(b, k, oc)][:],
                                     wout_sb[:, hg, oc, :], rhs,
                                     start=(c == 0), stop=(c == 1))
                    cnt[(b, k, oc)] = c + 1
                return
            if dve_combine:
                # p-half d4 gathers -> permuted multiply into gu (l,p,dx) ->
                # in-place bf16 add tree over (p,dx) -> 2 out_proj matmuls
                gu = gup.tile([128, 256, 16, 4], BF16, tag="gu", bufs=1)
                gua = gu[:]
                if gather8:
                    # one 4096-idx gather per block: per-call ucode fixed
                    # cost dominates, so fewer larger calls win even with
                    # bufs=1 (next gather waits this block's 4 mults)
                    g8 = gathp.tile([128, 4096, 4], BF16, tag="g8", bufs=1)
                    if stub_gather:
                        nc.gpsimd.ap_gather(
                            g8[:, 0:16, :],
                            quad[:].rearrange("p a b -> p (a b)"),
                            idxw[:, 0:1], channels=128,
                            num_elems=L, d=4, num_idxs=16)
                    else:
                        nc.gpsimd.ap_gather(
                            g8[:], quad[:].rearrange("p a b -> p (a b)"),
                            idxw[:], channels=128,
                            num_elems=L, d=4, num_idxs=4096)
                    ghs = [g8, g8]
                else:
                    ghs = []
                    for jh in range(2):
                        gh = gathp.tile([128, 2048, 4], BF16, tag="g4h")
                        ghs.append(gh)
                        if stub_gather:
                            nc.gpsimd.ap_gather(
                                gh[:, 0:16, :],
                                quad[:].rearrange("p a b -> p (a b)"),
                                idxw[:, 0:1], channels=128,
                                num_elems=L, d=4, num_idxs=16)
                        else:
                            nc.gpsimd.ap_gather(
                                gh[:], quad[:].rearrange("p a b -> p (a b)"),
                                idxw[:, jh * 128:(jh + 1) * 128],
                                channels=128, num_elems=L, d=4,
                                num_idxs=2048)
                for jq in range(4):
                    ubc = ubcp.tile([128, 4096], BF16, tag="ubc")
                    for hh in range(4):
                        r = ubc_rr[0] % 2
                        ubc_rr[0] += 1
                        dmaq = (dmas, dmaa)[r]
                        if stub_ubc and hh > 0:
                            continue
                        dmaq(ubc[hh * 32:(hh + 1) * 32, :],
                             AP(tensor=ud,
                                offset=(b * 128 + (hg * 4 + hh) * 16
                                        + jq * 4) * 2048 + k * 1024,
                                ap=[[0, 32], [2048, 4], [1, 1024]]))
                    gh = ghs[jq // 2]
                    jo = jq if gather8 else jq % 2
                    sl = gh[:, jo * 1024:(jo + 1) * 1024, :]
                    nc.vector.tensor_tensor(
                        out=AP(tensor=gua.tensor,
                               offset=gua.offset + jq * 16,
                               ap=[gua.ap[0], [4, 4], [64, 256], [1, 4]]),
                        in0=sl.rearrange("p a b -> p (a b)"),
                        in1=ubc[:], op=Alu.mult)

                def tadd(o, i1):
                    nc.vector.tensor_tensor(out=o, in0=o, in1=i1, op=Alu.add)
                tadd(gu[:, :, :, 0:2], gu[:, :, :, 2:4])
                tadd(gu[:, :, 0:8, 0:2], gu[:, :, 8:16, 0:2])
                tadd(gu[:, :, 0:4, 0:2], gu[:, :, 4:8, 0:2])
                tadd(gu[:, :, 0:2, 0:2], gu[:, :, 2:4, 0:2])
                tadd(gu[:, :, 0:1, 0:2], gu[:, :, 1:2, 0:2])
                tadd(gu[:, :, 0:1, 0:1], gu[:, :, 0:1, 1:2])
                rhs = AP(tensor=gua.tensor, offset=gua.offset,
                         ap=[gua.ap[0], [64, 256]])
                for oc in range(2):
                    c = cnt[(b, k, oc)]
                    nc.tensor.matmul(pso[(b, k, oc)][:],
                                     wout_sb[:, hg, oc, :], rhs,
                                     start=(c == 0), stop=(c == 1))
                    cnt[(b, k, oc)] = c + 1
                return
            g = gathp.tile([128, 4096, 4], BF16, tag="g4")
            if gather_f32:
                # f32-pair view: one gathered "element" = 2 packed bf16 taps,
                # so d=2 f32 halves the gather's free-size cost.
                qf = quad[:].rearrange("p a b -> p (a b)").bitcast(F32)
                if stub_gather:
                    nc.gpsimd.ap_gather(
                        g[:, 0:16, :].rearrange(
                            "p a b -> p (a b)").bitcast(F32),
                        qf, idxw[:, 0:1], channels=128,
                        num_elems=L, d=2, num_idxs=16)
                else:
                    nc.gpsimd.ap_gather(
                        g[:].rearrange("p a b -> p (a b)").bitcast(F32),
                        qf, idxw[:], channels=128,
                        num_elems=L, d=2, num_idxs=4096)
            elif stub_gather:
                nc.gpsimd.ap_gather(
                    g[:, 0:16, :], quad[:].rearrange("p a b -> p (a b)"),
                    idxw[:, 0:1], channels=128,
                    num_elems=L, d=4, num_idxs=16)
            else:
                nc.gpsimd.ap_gather(
                    g[:], quad[:].rearrange("p a b -> p (a b)"),
                    idxw[:], channels=128,
                    num_elems=L, d=4, num_idxs=4096)
            nj, pq = (2, 8) if ubc_half else (4, 4)
            for jq in range(nj):      # u-broadcasts: p in [pq*jq, pq*(jq+1))
                ubc = ubcp.tile([128, 1024 * pq], BF16, tag="ubc",
                                bufs=1 if ubc_half else 2)
                for hh in range(4):
                    # balance broadcast traffic across SP/ACT/Pool DMA queues
                    r = ubc_rr[0] % 2
                    ubc_rr[0] += 1
                    dmaq = (dmas, dmaa)[r]
                    if stub_ubc and hh > 0:
                        continue
                    dmaq(ubc[hh * 32:(hh + 1) * 32, :],
                         AP(tensor=ud,
                            offset=(b * 128 + (hg * 4 + hh) * 16 + jq * pq)
                            * 2048 + k * 1024,
                            ap=[[0, 32], [2048, pq], [1, 1024]]))
                sl = g[:, jq * pq * 256:(jq + 1) * pq * 256, :].rearrange(
                    "p a b -> p (a b)")
                nc.vector.tensor_tensor(out=sl, in0=sl, in1=ubc[:],
                                        op=Alu.mult)
            gap = g[:]
            for oc in range(2):
                for p in range(16):
                    for dxi in range(4):
                        rhs = AP(tensor=gap.tensor,
                                 offset=gap.offset + p * 1024 + dxi,
                                 ap=[gap.ap[0], [4, 256]])
                        c = cnt[(b, k, oc)]
                        nc.tensor.matmul(
                            pso[(b, k, oc)][:],
                            wout_sb[:, hg, oc, :], rhs,
                            start=(c == 0), stop=(c == 127))
                        cnt[(b, k, oc)] = c + 1

        def finalize(b, k):
            for oc in range(2):
                o_sb = outp.tile([128, 256], F32, tag="osb")
                nc.scalar.activation(out=o_sb[:], in_=pso[(b, k, oc)][:],
                                     func=Act.Identity,
                                     bias=obias_sb[:, oc:oc + 1], scale=1.0)
                dmaa(AP(tensor=out_e,
                        offset=((b * 2 + oc) * 128) * LSH + k * 256,
                        ap=[[LSH, 128], [1, 256]]), o_sb[:])

        # ---------------- emission schedule ----------------
        # idx path + hg0 quad first so the Pool engine's first gather can
        # start as early as possible; everything else fills in under the
        # gather stream.
        prep_fsh(0)
        prep_a(0)
        prep_c(0)
        prep_cs(0, 0)
        conv_sec(0, (0,))
        prep_b(0)
        prep_d(0)
        prep_cs(0, 1)
        gblock(0, 0, 0)
        conv_sec(0, (1,))
        gblock(0, 0, 1)
        prep_fsh(1)
        prep_a(1)
        prep_b(1)
        prep_c(1)
        prep_d(1)
        conv_sec(1, (0,))
        gblock(0, 1, 0)
        finalize(0, 0)
        prep_cs(1, 0)
        gblock(0, 1, 1)
        finalize(0, 1)
        prep_cs(1, 1)
        conv_sec(1, (1,))
        gblock(1, 0, 0)
        gblock(1, 0, 1)
        gblock(1, 1, 0)
        finalize(1, 0)
        gblock(1, 1, 1)
        finalize(1, 1)

    nc.compile()
    _GRAPH_CACHE[key] = nc
    return nc


def stage_inputs(inputs, core):
    """Build the per-core in_map (all arrays pre-laid-out for plain DMAs)."""
    bf16 = ml_dtypes.bfloat16
    feat = np.ascontiguousarray(
        np.asarray(inputs['feat_sd'], np.float32).reshape(B, C, L))
    lo = core * LSH
    WvT = np.asarray(inputs['value_proj_w'], np.float32).T.copy()
    WoffT = np.asarray(inputs['anchor_deform_w'], np.float32).T.copy()
    WattT = np.asarray(inputs['anchor_att_w'], np.float32).T.copy()
    WszT = np.asarray(inputs['size_deform_w'], np.float32).T.copy()
    WoutT = np.asarray(inputs['out_proj_w'], np.float32).T.copy()
    boff = np.asarray(inputs['anchor_deform_b'], np.float32)
    bsz = np.asarray(inputs['size_deform_b'], np.float32)
    bv = np.asarray(inputs['value_proj_b'], np.float32)
    bn_s = (np.asarray(inputs['bn_gamma'], np.float32)
            / np.sqrt(np.float32(1.0 + 1e-5)))
    beta = np.asarray(inputs['bn_beta'], np.float32)
    WoutT_sc = WoutT * bn_s[None, :]
    obias = (bv @ WoutT_sc + beta).reshape(2, 128).T
    sel8 = np.zeros((8, 128), np.float32)
    for h in range(8):
        sel8[h, h * 16:(h + 1) * 16] = float(W - 1)
    cols = (np.arange(W) + 0.5) / (W + EPS)
    rows = (np.arange(H) + 0.5) / (H + EPS)
    cx = np.tile(cols, H)[lo:lo + LSH].astype(np.float32)
    cy = np.repeat(rows, W)[lo:lo + LSH].astype(np.float32)
    cen2 = np.stack([np.broadcast_to(cx, (128, LSH)),
                     np.broadcast_to(cy, (128, LSH))], axis=1) * (W - 1.0)
    # woff/wsz packed: xy-interleaved output channels split into x|y planes
    woff = np.stack([WoffT[:, 0::2], WoffT[:, 1::2]],
                    axis=1)                      # [256, 2, 128]
    woff_t = woff.reshape(2, 128, 2, 128).transpose(1, 2, 0, 3)
    wsz = np.stack([WszT[:, 0::2], WszT[:, 1::2]], axis=1)  # [256, 2, 8]
    wsz_t = wsz.reshape(2, 128, 2, 8).transpose(1, 2, 0, 3)
    fr = feat.reshape(B, 2, 128, L)
    m = {
        'featb': np.ascontiguousarray(
            fr.transpose(2, 0, 1, 3)).astype(bf16),
        'fsh': np.ascontiguousarray(
            fr[:, :, :, lo:lo + LSH].transpose(2, 0, 1, 3)),
        'wv_t': np.ascontiguousarray(
            WvT.reshape(2, 128, 2, 128).transpose(1, 2, 0, 3)).astype(bf16),
        'woff_t': np.ascontiguousarray(woff_t),
        'boff_p': np.ascontiguousarray(
            np.stack([boff[0::2], boff[1::2]], axis=1)),
        'wsz_t': np.ascontiguousarray(wsz_t),
        'bsz_p': np.ascontiguousarray(
            np.stack([bsz[0::2], bsz[1::2]], axis=1)),
        'sel8': sel8,
        'watt_t': np.ascontiguousarray(
            WattT.reshape(2, 128, 128).transpose(1, 0, 2)),
        'batt_r': np.asarray(inputs['anchor_att_b'],
                             np.float32).reshape(1, 128),
        'ones1': np.ones((1, 128), np.float32),
        'ident': np.eye(128, dtype=np.float32),
        'wout_t': np.ascontiguousarray(
            WoutT_sc.reshape(2, 128, 2, 128).transpose(1, 0, 2, 3)
        ).astype(bf16),
        'obias': np.ascontiguousarray(obias),
        'cen2': np.ascontiguousarray(cen2),
    }
    return m


def kernel(**inputs):
    nc = build_graph()
    in_maps = [stage_inputs(inputs, i) for i in range(NCORES)]
    res = run_bass_kernel_spmd(nc, in_maps, core_ids=list(range(NCORES)))
    shards = [res.results[i]['out'].reshape(B, C, LSH) for i in range(NCORES)]
    full = np.concatenate(shards, axis=2).reshape(B, C, H, W)
    return full.astype(np.float32)



# revision 51
# speedup vs baseline: 3.5130x; 1.5993x over previous
"""Trainium2 Bass kernel for nn_AnchorDeformAtt (deformable anchor attention).

Sharding: spatial L-shard across 8 cores -- core i handles pixels
l in [512i, 512(i+1)) for BOTH batches and ALL heads. Zero collectives;
the host concatenates per-core output shards.

Design (the gather is ~75% of runtime; everything else hides under it):
  - Memory: quad rows (m[j], m[j+1], m[j+64], m[j+65]) bf16, so ONE
    index per sample point fetches all 4 bilinear taps (d=4 bf16 --
    measured faster on HW than the equivalent d=2 f32-pair view).
  - The GPSIMD engine runs ONLY ap_gathers: Pool-issued DMAs go through
    SWDGE holding the engine and head-of-line-block the gather queue
    (measured ~370us); all DMA issue lives on the SP/ACT queues.
  - Value conv in bf16 (feat staged bf16 host-side); bias folded out
    (softmax x bilinear weights sum to 1 => Wout @ bv is a constant
    output bias, merged with bn_beta host-side; BN scale folded into
    Wout). Prep convs in float32r (tf32 rate).
  - Gather stream (pwrap): the 16-partition wrap dim is the sample
    POINT p (j%16 = p), not l%16. The idx rows are then exactly fi's
    (h,p) partition rows: 8 SBUF->SBUF row-replication DMAs per (b,hg)
    replace the whole DRAM transpose round-trip (whose 2B-scattered
    stores cost 7.2us each and stalled the fill through DMA-counter
    aliasing). Bonus: gather output lands directly in (l,p,dx)-major
    order, so the u-multiply runs IN-PLACE on the gathered tile (no
    permuted write, no separate gu tile) -- the freed 32KB enables one
    4096-idx gather per block with double buffering (8 calls instead
    of 16; measured -360us total vs the DRAM-wrap 16-call version).
  - f32->i16 index conversion ROUNDS on HW (CoreSim truncates): the
    is_gt/subtract pass after the copy restores floor semantics.
  - u = attn * bilinear weights staged as [b][(h,p)][l][dx4],
    broadcast to each head's 32 channels with 3-dim-AP DMAs (cheap:
    fully hidden under the gather once off the Pool queue).
  - Combine (dve_combine): the gathered [128ch, l, p, dx] tile is
    multiplied in-place by u per p-quarter (strided 4-dim in1 AP reads
    the p-major ubc tile in (l,p,dx) order), then an in-place bf16
    pairwise add tree reduces (p,dx) -- 7 DVE ops replace 128
    accumulating matmuls; out_proj is 2 matmuls per gblock (stride-64
    rhs on the reduced column). ~200us faster on HW than PE-combine.
  - Emission schedule: idx path + hg0 quad first so the first gather
    starts as early as possible; conv_sec(1,(0,)) is emitted before
    gblock(0,1,0) so b1's quad copies queue on DVE ahead of the
    gather-dependent mults. conv_sec(1,(1,)) must stay after
    gblock(0,1,1): its memset waits on quad01's buffer, whose release
    needs that block's gathers (earlier emission deadlocks the DVE
    queue). Remaining timeline-model Pool gaps (~25us before the hg
    transition, ~21us at the b transition, ~33us drain) persist under
    every emission permutation tried (c01_early flag, act_all copy
    rebalancing) -- they come from combine-chain latency + quad buffer
    recycling (quads have only 2 buffers; a b1 quad build cannot start
    until the b0 gathers release its buffer), not emission order.
  - Measured dead ends (same-process A/B on HW): d=2 f32-pair gather
    (+100us despite half the elements), coarser ubc tiles with bufs=1
    (+200us), one 4096-idx gather per block with gathp bufs=1 (+220us:
    the mult-drain serialization outweighs the ~13us/call fixed cost),
    ubc issue pinned to one queue, and consts alternated onto ACT
    (both regress the fill in the timeline model).
"""
from contextlib import ExitStack

import numpy as np
import ml_dtypes

import concourse.bass as bass
import concourse.mybir as mybir
import concourse.tile as tile
from concourse import bacc
from concourse.bass_utils import run_bass_kernel_spmd

NH, NP = 8, 16
B, C, H, W = 2, 256, 64, 64
L = H * W            # 4096
NCORES = 8
LSH = L // NCORES    # 512
LPAD = L + 64        # pairs-memory rows (y1 tap reads idx+64)
EPS = 1e-6
F32 = mybir.dt.float32
BF16 = mybir.dt.bfloat16
I16 = mybir.dt.int16
F32R = mybir.dt.float32r
FP8 = mybir.dt.float8e4

_GRAPH_CACHE = {}

PARAM_SPECS = {
    'featb': ([128, B, 2, L], BF16),
    'fsh': ([128, B, 2, LSH], F32R),
    'wv_t': ([128, 2, 2, 128], BF16),    # K, hg, kc, M
    'woff_t': ([128, 2, 2, 128], F32R),   # K, xy, kc, M
    'boff_p': ([128, 2], F32),           # per-partition bias, xy
    'wsz_t': ([128, 2, 2, 8], F32R),      # K, xy, kc, 8
    'bsz_p': ([8, 2], F32),
    'sel8': ([8, 128], F32),             # sel8[h, h*16+p] = 1
    'watt_t': ([128, 2, 128], F32R),      # K, kc, N
    'batt_r': ([1, 128], F32),
    'ones1': ([1, 128], F32),
    'ident': ([128, 128], F32),
    'wout_t': ([128, 2, 2, 128], BF16),  # K, hg, oc, M (bn-scaled)
    'obias': ([128, 2], F32),            # (Wout_sc @ bv + beta) as [m, oc]
    'cen2': ([128, 2, LSH], F32),        # packed (x|y) centers
}


def build_graph(stub_gather=False, gather_f32=False, ubc_half=False,
                dve_combine=True, stub_ubc=False, gather8=False,
                qsplit=False, pwrap=True, c01_early=False):
    key = (stub_gather, gather_f32, ubc_half, dve_combine, stub_ubc, gather8,
           qsplit, pwrap, c01_early)
    if key in _GRAPH_CACHE:
        return _GRAPH_CACHE[key]

    nc = bacc.Bacc("TRN2", target_bir_lowering=False, debug=False,
                   num_devices=NCORES)
    dp = nc.declare_dram_parameter
    P = {n: dp(n, s, dt, isOutput=False) for n, (s, dt) in PARAM_SPECS.items()}
    out_e = dp("out", [B, 2, 128, LSH], F32, isOutput=True)

    # wrapped y0 idx staging, one tensor per (k, hg): [b][lq][hh][p][lhi]
    # (lq-major so the (hh,p) partition dims merge to one stride-16 dim)
    idxd = {(k, hg): nc.dram_tensor(f"idxd{k}{hg}", [B, 16, 4, 16, 16], I16)
            for k in range(2) for hg in range(2)}
    # u staging: [b][(h,p) 128][tap 2][l 512][dx 2]
    ud = nc.dram_tensor("ud", [B, 128, 2, LSH, 2], BF16)

    AP = bass.AP
    Act = mybir.ActivationFunctionType
    Alu = mybir.AluOpType

    with tile.TileContext(nc) as tc, ExitStack() as ctx:
        consts = ctx.enter_context(tc.tile_pool(name="consts", bufs=1))
        featp = ctx.enter_context(tc.tile_pool(name="featp", bufs=2))
        fshp = ctx.enter_context(tc.tile_pool(name="fshp", bufs=1))
        memp = ctx.enter_context(tc.tile_pool(name="memp", bufs=1))
        prep = ctx.enter_context(tc.tile_pool(name="prep", bufs=1))
        gm = ctx.enter_context(tc.tile_pool(name="gm", bufs=1))
        idxwp = ctx.enter_context(tc.tile_pool(name="idxwp", bufs=2))
        ubcp = ctx.enter_context(tc.tile_pool(name="ubcp", bufs=2))
        gathp = ctx.enter_context(tc.tile_pool(name="gathp", bufs=2))
        gup = ctx.enter_context(tc.tile_pool(name="gup", bufs=1))
        outp = ctx.enter_context(tc.tile_pool(name="outp", bufs=2))
        ps_v = ctx.enter_context(tc.tile_pool(name="ps_v", bufs=2, space="PSUM"))
        ps_p = ctx.enter_context(tc.tile_pool(name="ps_p", bufs=2, space="PSUM"))
        ps_o = ctx.enter_context(tc.tile_pool(name="ps_o", bufs=1, space="PSUM"))

        def dmas(out, in_):          # SP queue: consts, ft, idx stores, ubc
            nc.sync.dma_start(out=out, in_=in_)

        def dmaa(out, in_):          # ACT queue: fsh, idxw loads, uq, out
            nc.scalar.dma_start(out=out, in_=in_)

        # ---------------- constants ----------------
        crr = [0]

        def cload(name, q=None):
            shape, dt = PARAM_SPECS[name]
            t = consts.tile(list(shape), dt, tag=name, name=f"c_{name}")
            if q is None:
                q = (dmas, dmaa)[crr[0] % 2] if qsplit else dmas
                crr[0] += 1
            q(t[:], P[name].ap())
            return t

        def dmag(out, in_):
            nc.gpsimd.dma_start(out=out, in_=in_)

        wv_sb = cload('wv_t')
        woff_sb = cload('woff_t')
        boff_sb = cload('boff_p')
        wsz_sb = cload('wsz_t')
        bsz_sb = cload('bsz_p')
        sel8_sb = cload('sel8')
        watt_sb = cload('watt_t')
        batt_sb = cload('batt_r')
        ones_sb = cload('ones1')
        id_sb = cload('ident')
        wout_sb = cload('wout_t')
        obias_sb = cload('obias')
        cen_sb = cload('cen2')

        mem_t = {}
        st = {}

        # ---------------- value conv (bf16 quads, no bias) ----------------
        def conv_sec(b, hgs=(0, 1), act_all=False):
            qs = {}
            for hg in hgs:
                q = memp.tile([128, L, 4], BF16, tag="quad",
                              name=f"quad{b}{hg}", bufs=2)
                mem_t[(b, hg)] = q
                nc.vector.memset(q[:, L - 65:, :], 0.0)
                qs[hg] = q
            for c in range(4):
                ft = featp.tile([128, 2, 1024], BF16, tag="ft")
                ftq = (dmas, dmaa)[c % 2]
                ftq(ft[:], AP(tensor=P['featb'], offset=b * 2 * L + c * 1024,
                              ap=[[B * 2 * L, 128], [L, 2], [1, 1024]]))
                for j in range(2):
                    n = c * 2 + j
                    for hg in hgs:
                        q = qs[hg]
                        ps = ps_v.tile([128, 512], F32, tag="pv")
                        for kc in range(2):
                            nc.tensor.matmul(ps[:], wv_sb[:, hg, kc, :],
                                             ft[:, kc, j * 512:(j + 1) * 512],
                                             start=(kc == 0), stop=(kc == 1))
                        for dxi, sh in ((0, 0), (1, 1), (2, 64), (3, 65)):
                            if sh == 0:
                                o, i = q[:, n * 512:(n + 1) * 512, 0], ps[:]
                            elif n == 0:
                                o, i = q[:, 0:512 - sh, dxi], ps[:, sh:512]
                            else:
                                o = q[:, n * 512 - sh:(n + 1) * 512 - sh, dxi]
                                i = ps[:]
                            if dxi < 2 or act_all:
                                nc.scalar.activation(out=o, in_=i,
                                                     func=Act.Copy)
                            else:
                                nc.vector.tensor_copy(out=o, in_=i)

        # ---------------- prep phases (per b) ----------------
        def prep_fsh(b):
            s = st.setdefault(b, {})
            fsh = fshp.tile([128, 2, LSH], F32R, tag="fsh")
            dmaa(fsh[:], P['fsh'].ap()[:, b, :, :])
            s['fsh'] = fsh

        def prep_a(b):
            """offset+size convs -> packed offp/szbp [128,2,LSH]."""
            s = st[b]
            fsh = s['fsh']
            offp = prep.tile([128, 2, LSH], F32, tag="offp")
            szbp = prep.tile([128, 2, LSH], F32, tag="szbp")
            s['offp'], s['szbp'] = offp, szbp
            for xy in range(2):
                ps = ps_p.tile([128, 512], F32, tag="pp", name="psz")
                for kc in range(2):
                    nc.tensor.matmul(ps[0:8, :], wsz_sb[:, xy, kc, :],
                                     fsh[:, kc, :], start=(kc == 0),
                                     stop=(kc == 1))
                szs = gm.tile([8, LSH], F32, tag="szs")
                nc.scalar.activation(out=szs[:], in_=ps[0:8, :],
                                     func=Act.Sigmoid,
                                     bias=bsz_sb[:, xy:xy + 1], scale=1.0)
                nc.vector.tensor_scalar(out=szs[:], in0=szs[:], scalar1=0.75,
                                        scalar2=0.25, op0=Alu.min, op1=Alu.max)
                psb = ps_p.tile([128, 512], F32, tag="pp", name="psb")
                nc.tensor.matmul(psb[:], sel8_sb[:], szs[:],
                                 start=True, stop=True)
                nc.vector.tensor_copy(out=szbp[:, xy, :], in_=psb[:])
                ps2 = ps_p.tile([128, 512], F32, tag="pp", name="po")
                for kc in range(2):
                    nc.tensor.matmul(ps2[:], woff_sb[:, xy, kc, :],
                                     fsh[:, kc, :], start=(kc == 0),
                                     stop=(kc == 1))
                nc.scalar.activation(out=offp[:, xy, :], in_=ps2[:],
                                     func=Act.Sigmoid,
                                     bias=boff_sb[:, xy:xy + 1], scale=1.0)

        def prep_c(b):
            """grid -> floor -> flat y0 idx (packed x|y in one [128,2,LSH])."""
            s = st[b]
            offp, szbp = s['offp'], s['szbp']
            o2 = offp[:].rearrange("p a b -> p (a b)")
            s2 = szbp[:].rearrange("p a b -> p (a b)")
            cf = gm.tile([128, 2, LSH], F32, tag="cf")
            c2 = cf[:].rearrange("p a b -> p (a b)")
            ci = gm.tile([128, 2, LSH], I16, tag="ci")
            i2 = ci[:].rearrange("p a b -> p (a b)")
            msk = gm.tile([128, 2, LSH], F32, tag="msk")
            m2 = msk[:].rearrange("p a b -> p (a b)")
            nc.vector.tensor_scalar(out=o2, in0=o2, scalar1=-0.5,
                                    scalar2=None, op0=Alu.add)
            nc.vector.tensor_tensor(out=o2, in0=o2, in1=s2, op=Alu.mult)
            nc.vector.tensor_tensor(
                out=o2, in0=o2,
                in1=cen_sb[:].rearrange("p a b -> p (a b)"), op=Alu.add)
            nc.vector.tensor_scalar(out=o2, in0=o2, scalar1=float(W - 1),
                                    scalar2=0.0, op0=Alu.min, op1=Alu.max)
            # f32->i16 conversion rounds to nearest on HW: compare-and-
            # subtract corrects round-up cases back to floor
            nc.vector.tensor_copy(out=i2, in_=o2)
            nc.vector.tensor_copy(out=c2, in_=i2)
            nc.vector.tensor_tensor(out=m2, in0=c2, in1=o2, op=Alu.is_gt)
            nc.vector.tensor_tensor(out=c2, in0=c2, in1=m2, op=Alu.subtract)
            nc.vector.tensor_tensor(out=o2, in0=o2, in1=c2, op=Alu.subtract)
            # flat y0 = y0f*W + x0f -> reuse szbp x-half as scratch, fi -> ci
            fl = szbp[:, 0, :]
            nc.vector.tensor_scalar(out=fl, in0=cf[:, 1, :],
                                    scalar1=float(W), scalar2=None,
                                    op0=Alu.mult)
            nc.vector.tensor_tensor(out=fl, in0=fl, in1=cf[:, 0, :],
                                    op=Alu.add)
            fi = gm.tile([128, LSH], I16, tag="fi")
            nc.vector.tensor_copy(out=fi[:], in_=fl)
            s['fi'] = fi
            # wx/wy in offp halves; cf/msk slots free for prep_d reuse
            s['cf'], s['msk'] = cf, msk

        def prep_cs(b, k, hgs=(0, 1)):
            """Wrapped y0 idx stores for l-block k + clean idxw loads +
            DVE-derived y1 idx tiles."""
            s = st[b]
            fi = s['fi']
            if pwrap:
                # p-wrapped idx streams (j%16 = point p): the idx rows ARE
                # fi's (h,p) partition rows -- pure SBUF->SBUF row
                # replication, no DRAM round-trip. One tile per (b,hg)
                # covers both k-blocks.
                if k != 0:
                    return
                for hg in hgs:
                    ix = idxwp.tile([128, 512], I16, tag=f"ixp{hg}",
                                    name=f"ixp{b}{hg}")
                    st[('ix', b, hg)] = ix
                    for hh in range(4):
                        for dup in range(2):
                            r = hh * 32 + dup * 16
                            dmaa(ix[r:r + 16, :],
                                 fi[hg * 64 + hh * 16:hg * 64 + hh * 16 + 16,
                                    :])
                return
            for hg in hgs:
                # one store for the whole head-group: partitions (hh,p) merge
                # to a single stride-16 dim in [b][lq][hh][p][lhi]
                dmas(AP(tensor=idxd[(k, hg)], offset=b * 16384,
                        ap=[[16, 64], [1, 16], [1024, 16]]),
                     fi[hg * 64:(hg + 1) * 64, k * 256:(k + 1) * 256])
                ix = idxwp.tile([128, 256], I16, tag=f"ix{hg}{k}",
                                name=f"ix{b}{hg}{k}")
                st[('ix', b, hg, k)] = ix
                for hh in range(4):
                    # 32 contiguous partitions (dup 2 x lq 16) per load; the
                    # dup replication is a stride-0 source dim
                    dmaa(ix[hh * 32:(hh + 1) * 32, :],
                         AP(tensor=idxd[(k, hg)],
                            offset=b * 16384 + hh * 256,
                            ap=[[0, 2], [1024, 16], [1, 256]]))

        def prep_b(b):
            """attn conv (pixel-major) + softmax + transpose -> aT."""
            s = st[b]
            fsh = s['fsh']
            aT = prep.tile([128, LSH], F32, tag="aT")
            s['aT'] = aT
            for lb in range(LSH // 128):
                ps = ps_p.tile([128, 128], F32, tag="pp", name="pa")
                for kc in range(2):
                    nc.tensor.matmul(ps[:], fsh[:, kc, lb * 128:(lb + 1) * 128],
                                     watt_sb[:, kc, :], start=(kc == 0),
                                     stop=False)
                nc.tensor.matmul(ps[:], ones_sb[:], batt_sb[:],
                                 start=False, stop=True)
                ae = gm.tile([128, 8, 16], F32, tag="ae")
                nc.scalar.activation(out=ae[:], in_=ps[:], func=Act.Exp)
                ssum = gm.tile([128, 8, 1], F32, tag="ssum")
                nc.vector.tensor_reduce(out=ssum[:], in_=ae[:],
                                        axis=mybir.AxisListType.X, op=Alu.add)
                nc.vector.reciprocal(out=ssum[:], in_=ssum[:])
                ssa = ssum[:]
                nc.vector.tensor_tensor(
                    out=ae[:], in0=ae[:],
                    in1=AP(tensor=ssa.tensor, offset=ssa.offset,
                           ap=[ssa.ap[0], [1, 8], [0, 16]]),
                    op=Alu.mult)
                pst = ps_p.tile([128, 128], F32, tag="pp", name="pt")
                nc.tensor.transpose(pst[:], ae[:].rearrange("p a b -> p (a b)"),
                                    id_sb[:])
                nc.scalar.activation(out=aT[:, lb * 128:(lb + 1) * 128],
                                     in_=pst[:], func=Act.Copy)

        def prep_d(b):
            """u = attn * bilinear -> upair [128, tap, l, dx] -> DRAM."""
            s = st[b]
            offp, cf, msk, aT = s['offp'], s['cf'], s['msk'], s['aT']
            wx, wy = offp[:, 0, :], offp[:, 1, :]
            omx, omy = msk[:, 0, :], msk[:, 1, :]
            ay0, ay1 = cf[:, 0, :], cf[:, 1, :]
            nc.vector.tensor_scalar(out=omx, in0=wx, scalar1=-1.0,
                                    scalar2=1.0, op0=Alu.mult, op1=Alu.add)
            nc.vector.tensor_scalar(out=omy, in0=wy, scalar1=-1.0,
                                    scalar2=1.0, op0=Alu.mult, op1=Alu.add)
            nc.vector.tensor_tensor(out=ay0, in0=aT[:], in1=omy, op=Alu.mult)
            nc.vector.tensor_tensor(out=ay1, in0=aT[:], in1=wy, op=Alu.mult)
            uq = gm.tile([128, LSH, 4], BF16, tag="uq")
            for dxi, (yf, xf) in enumerate(((ay0, omx), (ay0, wx),
                                            (ay1, omx), (ay1, wx))):
                nc.vector.tensor_tensor(out=uq[:, :, dxi], in0=yf,
                                        in1=xf, op=Alu.mult)
            dmaa(AP(tensor=ud, offset=b * 128 * 2048,
                    ap=[[2048, 128], [1, 2048]]),
                 uq[:].rearrange("p a b -> p (a b)"))

        # ---------------- gather + combine ----------------
        pso = {}
        cnt = {}
        ubc_rr = [0]

        def gblock(b, hg, k):
            quad = mem_t[(b, hg)]
            idxw = st[('ix', b, hg)] if pwrap else st[('ix', b, hg, k)]
            for oc in range(2):
                if (b, k, oc) not in pso:
                    pso[(b, k, oc)] = ps_o.tile([128, 256], F32,
                                                tag=f"po{k}{oc}",
                                                name=f"po{b}{k}{oc}")
                    cnt[(b, k, oc)] = 0
            if dve_combine and pwrap:
                # p-wrapped stream: gather output is ALREADY (l,p,dx)-major,
                # so one 4096-idx gather per block lands in-place; multiply
                # in-place per p-quarter, bf16 add tree, 2 out_proj matmuls
                gw = gathp.tile([128, 256, 16, 4], BF16, tag="gw")
                gwa = gw[:]
                idxw2 = st[('ix', b, hg)]
                if stub_gather:
                    nc.gpsimd.ap_gather(
                        AP(tensor=gwa.tensor, offset=gwa.offset,
                           ap=[gwa.ap[0], [1, 64]]),
                        quad[:].rearrange("p a b -> p (a b)"),
                        idxw2[:, 0:1], channels=128,
                        num_elems=L, d=4, num_idxs=16)
                else:
                    nc.gpsimd.ap_gather(
                        gw[:].rearrange("p a b c -> p (a b c)"),
                        quad[:].rearrange("p a b -> p (a b)"),
                        idxw2[:, k * 256:(k + 1) * 256],
                        channels=128, num_elems=L, d=4, num_idxs=4096)
                for jq in range(4):
                    ubc = ubcp.tile([128, 4096], BF16, tag="ubc")
                    for hh in range(4):
                        r = ubc_rr[0] % 2
                        ubc_rr[0] += 1
                        dmaq = (dmas, dmaa)[r]
                        if stub_ubc and hh > 0:
                            continue
                        dmaq(ubc[hh * 32:(hh + 1) * 32, :],
                             AP(tensor=ud,
                                offset=(b * 128 + (hg * 4 + hh) * 16
                                        + jq * 4) * 2048 + k * 1024,
                                ap=[[0, 32], [2048, 4], [1, 1024]]))
                    sl = gw[:, :, jq * 4:(jq + 1) * 4, :]
                    ubca = ubc[:]
                    nc.vector.tensor_tensor(
                        out=sl, in0=sl,
                        in1=AP(tensor=ubca.tensor, offset=ubca.offset,
                               ap=[ubca.ap[0], [4, 256], [1024, 4], [1, 4]]),
                        op=Alu.mult)

                def tadd2(o, i1):
                    nc.vector.tensor_tensor(out=o, in0=o, in1=i1, op=Alu.add)
                tadd2(gw[:, :, :, 0:2], gw[:, :, :, 2:4])
                tadd2(gw[:, :, 0:8, 0:2], gw[:, :, 8:16, 0:2])
                tadd2(gw[:, :, 0:4, 0:2], gw[:, :, 4:8, 0:2])
                tadd2(gw[:, :, 0:2, 0:2], gw[:, :, 2:4, 0:2])
                tadd2(gw[:, :, 0:1, 0:2], gw[:, :, 1:2, 0:2])
                tadd2(gw[:, :, 0:1, 0:1], gw[:, :, 0:1, 1:2])
                rhs = AP(tensor=gwa.tensor, offset=gwa.offset,
                         ap=[gwa.ap[0], [64, 256]])
                for oc in range(2):
                    c = cnt[(b, k, oc)]
                    nc.tensor.matmul(pso[(b, k, oc)][:],
                                     wout_sb[:, hg, oc, :], rhs,
                                     start=(c == 0), stop=(c == 1))
                    cnt[(b, k, oc)] = c + 1
                return
            if dve_combine:
                # p-half d4 gathers -> permuted multiply into gu (l,p,dx) ->
                # in-place bf16 add tree over (p,dx) -> 2 out_proj matmuls
                gu = gup.tile([128, 256, 16, 4], BF16, tag="gu", bufs=1)
                gua = gu[:]
                if gather8:
                    # one 4096-idx gather per block: per-call ucode fixed
                    # cost dominates, so fewer larger calls win even with
                    # bufs=1 (next gather waits this block's 4 mults)
                    g8 = gathp.tile([128, 4096, 4], BF16, tag="g8", bufs=1)
                    if stub_gather:
                        nc.gpsimd.ap_gather(
                            g8[:, 0:16, :],
                            quad[:].rearrange("p a b -> p (a b)"),
                            idxw[:, 0:1], channels=128,
                            num_elems=L, d=4, num_idxs=16)
                    else:
                        nc.gpsimd.ap_gather(
                            g8[:], quad[:].rearrange("p a b -> p (a b)"),
                            idxw[:], channels=128,
                            num_elems=L, d=4, num_idxs=4096)
                    ghs = [g8, g8]
                else:
                    ghs = []
                    for jh in range(2):
                        gh = gathp.tile([128, 2048, 4], BF16, tag="g4h")
                        ghs.append(gh)
                        if stub_gather:
                            nc.gpsimd.ap_gather(
                                gh[:, 0:16, :],
                                quad[:].rearrange("p a b -> p (a b)"),
                                idxw[:, 0:1], channels=128,
                                num_elems=L, d=4, num_idxs=16)
                        else:
                            nc.gpsimd.ap_gather(
                                gh[:], quad[:].rearrange("p a b -> p (a b)"),
                                idxw[:, jh * 128:(jh + 1) * 128],
                                channels=128, num_elems=L, d=4,
                                num_idxs=2048)
                for jq in range(4):
                    ubc = ubcp.tile([128, 4096], BF16, tag="ubc")
                    for hh in range(4):
                        r = ubc_rr[0] % 2
                        ubc_rr[0] += 1
                        dmaq = (dmas, dmaa)[r]
                        if stub_ubc and hh > 0:
                            continue
                        dmaq(ubc[hh * 32:(hh + 1) * 32, :],
                             AP(tensor=ud,
                                offset=(b * 128 + (hg * 4 + hh) * 16
                                        + jq * 4) * 2048 + k * 1024,
                                ap=[[0, 32], [2048, 4], [1, 1024]]))
                    gh = ghs[jq // 2]
                    jo = jq if gather8 else jq % 2
                    sl = gh[:, jo * 1024:(jo + 1) * 1024, :]
                    nc.vector.tensor_tensor(
                        out=AP(tensor=gua.tensor,
                               offset=gua.offset + jq * 16,
                               ap=[gua.ap[0], [4, 4], [64, 256], [1, 4]]),
                        in0=sl.rearrange("p a b -> p (a b)"),
                        in1=ubc[:], op=Alu.mult)

                def tadd(o, i1):
                    nc.vector.tensor_tensor(out=o, in0=o, in1=i1, op=Alu.add)
                tadd(gu[:, :, :, 0:2], gu[:, :, :, 2:4])
                tadd(gu[:, :, 0:8, 0:2], gu[:, :, 8:16, 0:2])
                tadd(gu[:, :, 0:4, 0:2], gu[:, :, 4:8, 0:2])
                tadd(gu[:, :, 0:2, 0:2], gu[:, :, 2:4, 0:2])
                tadd(gu[:, :, 0:1, 0:2], gu[:, :, 1:2, 0:2])
                tadd(gu[:, :, 0:1, 0:1], gu[:, :, 0:1, 1:2])
                rhs = AP(tensor=gua.tensor, offset=gua.offset,
                         ap=[gua.ap[0], [64, 256]])
                for oc in range(2):
                    c = cnt[(b, k, oc)]
                    nc.tensor.matmul(pso[(b, k, oc)][:],
                                     wout_sb[:, hg, oc, :], rhs,
                                     start=(c == 0), stop=(c == 1))
                    cnt[(b, k, oc)] = c + 1
                return
            g = gathp.tile([128, 4096, 4], BF16, tag="g4")
            if gather_f32:
                # f32-pair view: one gathered "element" = 2 packed bf16 taps,
                # so d=2 f32 halves the gather's free-size cost.
                qf = quad[:].rearrange("p a b -> p (a b)").bitcast(F32)
                if stub_gather:
                    nc.gpsimd.ap_gather(
                        g[:, 0:16, :].rearrange(
                            "p a b -> p (a b)").bitcast(F32),
                        qf, idxw[:, 0:1], channels=128,
                        num_elems=L, d=2, num_idxs=16)
                else:
                    nc.gpsimd.ap_gather(
                        g[:].rearrange("p a b -> p (a b)").bitcast(F32),
                        qf, idxw[:], channels=128,
                        num_elems=L, d=2, num_idxs=4096)
            elif stub_gather:
                nc.gpsimd.ap_gather(
                    g[:, 0:16, :], quad[:].rearrange("p a b -> p (a b)"),
                    idxw[:, 0:1], channels=128,
                    num_elems=L, d=4, num_idxs=16)
            else:
                nc.gpsimd.ap_gather(
                    g[:], quad[:].rearrange("p a b -> p (a b)"),
                    idxw[:], channels=128,
                    num_elems=L, d=4, num_idxs=4096)
            nj, pq = (2, 8) if ubc_half else (4, 4)
            for jq in range(nj):      # u-broadcasts: p in [pq*jq, pq*(jq+1))
                ubc = ubcp.tile([128, 1024 * pq], BF16, tag="ubc",
                                bufs=1 if ubc_half else 2)
                for hh in range(4):
                    # balance broadcast traffic across SP/ACT/Pool DMA queues
                    r = ubc_rr[0] % 2
                    ubc_rr[0] += 1
                    dmaq = (dmas, dmaa)[r]
                    if stub_ubc and hh > 0:
                        continue
                    dmaq(ubc[hh * 32:(hh + 1) * 32, :],
                         AP(tensor=ud,
                            offset=(b * 128 + (hg * 4 + hh) * 16 + jq * pq)
                            * 2048 + k * 1024,
                            ap=[[0, 32], [2048, pq], [1, 1024]]))
                sl = g[:, jq * pq * 256:(jq + 1) * pq * 256, :].rearrange(
                    "p a b -> p (a b)")
                nc.vector.tensor_tensor(out=sl, in0=sl, in1=ubc[:],
                                        op=Alu.mult)
            gap = g[:]
            for oc in range(2):
                for p in range(16):
                    for dxi in range(4):
                        rhs = AP(tensor=gap.tensor,
                                 offset=gap.offset + p * 1024 + dxi,
                                 ap=[gap.ap[0], [4, 256]])
                        c = cnt[(b, k, oc)]
                        nc.tensor.matmul(
                            pso[(b, k, oc)][:],
                            wout_sb[:, hg, oc, :], rhs,
                            start=(c == 0), stop=(c == 127))
                        cnt[(b, k, oc)] = c + 1

        def finalize(b, k):
            for oc in range(2):
                o_sb = outp.tile([128, 256], F32, tag="osb")
                nc.scalar.activation(out=o_sb[:], in_=pso[(b, k, oc)][:],
                                     func=Act.Identity,
                                     bias=obias_sb[:, oc:oc + 1], scale=1.0)
                dmaa(AP(tensor=out_e,
                        offset=((b * 2 + oc) * 128) * LSH + k * 256,
                        ap=[[LSH, 128], [1, 256]]), o_sb[:])

        # ---------------- emission schedule ----------------
        # idx path + hg0 quad first so the Pool engine's first gather can
        # start as early as possible; everything else fills in under the
        # gather stream.
        prep_fsh(0)
        prep_a(0)
        prep_c(0)
        prep_cs(0, 0)
        conv_sec(0, (0,))
        if c01_early:
            conv_sec(0, (1,))
        prep_b(0)
        prep_d(0)
        prep_cs(0, 1)
        gblock(0, 0, 0)
        if not c01_early:
            conv_sec(0, (1,))
        gblock(0, 0, 1)
        prep_fsh(1)
        prep_a(1)
        prep_b(1)
        prep_c(1)
        prep_d(1)
        conv_sec(1, (0,))
        gblock(0, 1, 0)
        finalize(0, 0)
        prep_cs(1, 0)
        gblock(0, 1, 1)
        finalize(0, 1)
        prep_cs(1, 1)
        conv_sec(1, (1,))
        gblock(1, 0, 0)
        gblock(1, 0, 1)
        gblock(1, 1, 0)
        finalize(1, 0)
        gblock(1, 1, 1)
        finalize(1, 1)

    nc.compile()
    _GRAPH_CACHE[key] = nc
    return nc


def stage_inputs(inputs, core):
    """Build the per-core in_map (all arrays pre-laid-out for plain DMAs)."""
    bf16 = ml_dtypes.bfloat16
    feat = np.ascontiguousarray(
        np.asarray(inputs['feat_sd'], np.float32).reshape(B, C, L))
    lo = core * LSH
    WvT = np.asarray(inputs['value_proj_w'], np.float32).T.copy()
    WoffT = np.asarray(inputs['anchor_deform_w'], np.float32).T.copy()
    WattT = np.asarray(inputs['anchor_att_w'], np.float32).T.copy()
    WszT = np.asarray(inputs['size_deform_w'], np.float32).T.copy()
    WoutT = np.asarray(inputs['out_proj_w'], np.float32).T.copy()
    boff = np.asarray(inputs['anchor_deform_b'], np.float32)
    bsz = np.asarray(inputs['size_deform_b'], np.float32)
    bv = np.asarray(inputs['value_proj_b'], np.float32)
    bn_s = (np.asarray(inputs['bn_gamma'], np.float32)
            / np.sqrt(np.float32(1.0 + 1e-5)))
    beta = np.asarray(inputs['bn_beta'], np.float32)
    WoutT_sc = WoutT * bn_s[None, :]
    obias = (bv @ WoutT_sc + beta).reshape(2, 128).T
    sel8 = np.zeros((8, 128), np.float32)
    for h in range(8):
        sel8[h, h * 16:(h + 1) * 16] = float(W - 1)
    cols = (np.arange(W) + 0.5) / (W + EPS)
    rows = (np.arange(H) + 0.5) / (H + EPS)
    cx = np.tile(cols, H)[lo:lo + LSH].astype(np.float32)
    cy = np.repeat(rows, W)[lo:lo + LSH].astype(np.float32)
    cen2 = np.stack([np.broadcast_to(cx, (128, LSH)),
                     np.broadcast_to(cy, (128, LSH))], axis=1) * (W - 1.0)
    # woff/wsz packed: xy-interleaved output channels split into x|y planes
    woff = np.stack([WoffT[:, 0::2], WoffT[:, 1::2]],
                    axis=1)                      # [256, 2, 128]
    woff_t = woff.reshape(2, 128, 2, 128).transpose(1, 2, 0, 3)
    wsz = np.stack([WszT[:, 0::2], WszT[:, 1::2]], axis=1)  # [256, 2, 8]
    wsz_t = wsz.reshape(2, 128, 2, 8).transpose(1, 2, 0, 3)
    fr = feat.reshape(B, 2, 128, L)
    m = {
        'featb': np.ascontiguousarray(
            fr.transpose(2, 0, 1, 3)).astype(bf16),
        'fsh': np.ascontiguousarray(
            fr[:, :, :, lo:lo + LSH].transpose(2, 0, 1, 3)),
        'wv_t': np.ascontiguousarray(
            WvT.reshape(2, 128, 2, 128).transpose(1, 2, 0, 3)).astype(bf16),
        'woff_t': np.ascontiguousarray(woff_t),
        'boff_p': np.ascontiguousarray(
            np.stack([boff[0::2], boff[1::2]], axis=1)),
        'wsz_t': np.ascontiguousarray(wsz_t),
        'bsz_p': np.ascontiguousarray(
            np.stack([bsz[0::2], bsz[1::2]], axis=1)),
        'sel8': sel8,
        'watt_t': np.ascontiguousarray(
            WattT.reshape(2, 128, 128).transpose(1, 0, 2)),
        'batt_r': np.asarray(inputs['anchor_att_b'],
                             np.float32).reshape(1, 128),
        'ones1': np.ones((1, 128), np.float32),
        'ident': np.eye(128, dtype=np.float32),
        'wout_t': np.ascontiguousarray(
            WoutT_sc.reshape(2, 128, 2, 128).transpose(1, 0, 2, 3)
        ).astype(bf16),
        'obias': np.ascontiguousarray(obias),
        'cen2': np.ascontiguousarray(cen2),
    }
    return m


def kernel(**inputs):
    nc = build_graph()
    in_maps = [stage_inputs(inputs, i) for i in range(NCORES)]
    res = run_bass_kernel_spmd(nc, in_maps, core_ids=list(range(NCORES)))
    shards = [res.results[i]['out'].reshape(B, C, LSH) for i in range(NCORES)]
    full = np.concatenate(shards, axis=2).reshape(B, C, H, W)
    return full.astype(np.float32)

